# revision 1
# baseline (speedup 1.0000x reference)
"""
2-layer GAT on Trainium2 (8 NeuronCores, SPMD via bass/Tile).

Sharding: destination nodes are block-sharded across the 8 cores (6250
nodes each).  All per-edge work runs on the core owning the edge's dst.
Layer-0 node-level compute (h = x @ W1pack) is replicated on every core
(cheap), avoiding collectives for layer 0.  Layer 1 needs layer-0
output of all nodes, so the pipeline is two bass kernels with a host
gather of per-core node tables in between.

All per-edge feature movement uses dma_gather (InstDMAGatherAnt):
  - table1 [N, 256] bf16 rows = [h(128) | d1(8) | s1(8) | pad]  (512 B)
    gathered by src.  int16 idx limit (32767) is handled by splitting
    every dst tile's chunks into "lo" (src < SPLIT) and "hi" chunks,
    gathered from base row 0 / row SPLIT respectively.
  - table_s [N, 128] bf16 rows = [s1(8) | pad] (256 B) gathered by dst
    via the pair-row view [N/2, 256] with idx = dst>>1 (fits int16),
    then an even/odd select picks cols 0:8 vs 128:136.
  - kernel B: table2 [N, 128] bf16 = [feat2(16) | d2 | s2 | pad],
    pair-row gathers by src and by dst + parity selects.

Aggregation per dst tile of 128 nodes: for each chunk of 128 edges an
S one-hot (S[e,j] = dstlocal[e]==j, built on DVE by iota compare) and
a PE matmul accumulate psum[dst,:] += S.T @ [feat*ex | ex]; then
normalize by the summed ex.  Softmax max-subtraction is skipped: the
attention logits here are O(0.3) so exp() is stable, and softmax is
shift-invariant.
"""

import os
import sys
from contextlib import ExitStack

import numpy as np
import ml_dtypes

for _p in ("/opt/trn_rl_repo",):
    if os.path.isdir(_p) and _p not in sys.path:
        sys.path.insert(0, _p)

import concourse.bass as bass
import concourse.bacc as bacc
import concourse.tile as tile
from concourse import mybir
from concourse import bass_utils
from concourse._compat import with_exitstack

F32 = mybir.dt.float32
BF16 = mybir.dt.bfloat16
I32 = mybir.dt.int32
I16 = mybir.dt.int16
AF = mybir.ActivationFunctionType
OP = mybir.AluOpType
P = 128
BF = ml_dtypes.bfloat16


class Cfg:
    def __init__(self, N, E, ncores, split=32768, neg=0.2, in_ch=128,
                 f=128, heads=8, hid=16, out=16):
        self.N = N
        self.E = E
        self.NCORES = ncores
        self.SPLIT = split
        self.NEG = neg
        self.IN = in_ch
        self.F = f
        self.H = heads
        self.HID = hid
        self.OUT = out
        assert N % ncores == 0
        self.NPC = N // ncores
        self.TPC = (self.NPC + P - 1) // P
        self.NPC_PAD = self.TPC * P
        self.NTILES = ncores * self.TPC
        self.N_PAD = self.NTILES * P
        self.NCHL_T = None
        self.NCHH_T = None
        self.NCH_T = None
        self.NCH = None


def _wrap16(vals):
    """[n] slot-ordered values -> [128, n//16] int16 wrapped layout."""
    n = vals.shape[0]
    assert n % 16 == 0
    w = vals.reshape(-1, 16).T.astype(np.int16)      # [16, n//16]
    return np.ascontiguousarray(np.tile(w, (8, 1)))  # [128, n//16]


def _prep_graph(cfg, edge_index):
    N, NPC, SPL = cfg.N, cfg.NPC, cfg.SPLIT
    src = np.concatenate([edge_index[0], np.arange(N, dtype=np.int64)])
    dst = np.concatenate([edge_index[1], np.arange(N, dtype=np.int64)])
    core = dst // NPC
    ld = dst - core * NPC
    tile_id = ld // P
    dstloc = ld % P
    hi = (src >= SPL).astype(np.int64)
    order = np.lexsort((src, hi, tile_id, core))
    src, dst, core, tile_id, dstloc, hi = (a[order] for a in
                                           (src, dst, core, tile_id,
                                            dstloc, hi))
    keyf = (core * cfg.TPC + tile_id) * 2 + hi
    cntf = np.bincount(keyf, minlength=cfg.NCORES * cfg.TPC * 2)
    cnt_lo = cntf[0::2].reshape(cfg.NCORES, cfg.TPC)
    cnt_hi = cntf[1::2].reshape(cfg.NCORES, cfg.TPC)
    cfg.NCHL_T = max(1, int(np.max((cnt_lo + P - 1) // P)))
    cfg.NCHH_T = max(1, int(np.max((cnt_hi + P - 1) // P)))
    cfg.NCH_T = cfg.NCHL_T + cfg.NCHH_T
    cfg.NCH = cfg.NCH_T * cfg.TPC
    starts = np.concatenate([[0], np.cumsum(cntf)])

    pc = dict(srcw_lo=[], srcw_hi=[], dstw=[], srcw=[], dstloc_f=[],
              dst_par=[], src_par=[])
    for c in range(cfg.NCORES):
        ns_lo = cfg.TPC * cfg.NCHL_T * P
        ns_hi = cfg.TPC * cfg.NCHH_T * P
        ns = cfg.NCH * P
        v_srclo = np.zeros(ns_lo, np.int64)
        v_srchi = np.zeros(ns_hi, np.int64)
        v_dst = np.zeros(ns, np.int64)          # dst>>1 per slot
        v_src = np.zeros(ns, np.int64)          # src>>1 per slot
        v_dstloc = np.full(ns, -1.0, np.float32)
        v_dpar = np.ones(ns, np.float32)
        v_spar = np.ones(ns, np.float32)
        for t in range(cfg.TPC):
            for h in (0, 1):
                k = ((c * cfg.TPC + t) * 2 + h)
                n = int(cntf[k])
                if n == 0:
                    continue
                sl = slice(starts[k], starts[k] + n)
                e_src = src[sl]
                e_dst = dst[sl]
                e_dl = dstloc[sl]
                pos = np.arange(n)
                if h == 0:
                    v_srclo[t * cfg.NCHL_T * P + pos] = e_src
                    ch = t * cfg.NCH_T + pos // P
                else:
                    v_srchi[t * cfg.NCHH_T * P + pos] = e_src - SPL
                    ch = t * cfg.NCH_T + cfg.NCHL_T + pos // P
                slot = ch * P + pos % P
                v_dst[slot] = e_dst >> 1
                v_src[slot] = e_src >> 1
                v_dstloc[slot] = e_dl
                v_dpar[slot] = 1.0 - (e_dst & 1)
                v_spar[slot] = 1.0 - (e_src & 1)
        pc["srcw_lo"].append(_wrap16(v_srclo))
        pc["srcw_hi"].append(_wrap16(v_srchi))
        pc["dstw"].append(_wrap16(v_dst))
        pc["srcw"].append(_wrap16(v_src))
        # slot arrays in [128, NCH] layout: slot = ch*128 + p -> [p, ch]
        pc["dstloc_f"].append(
            np.ascontiguousarray(v_dstloc.reshape(cfg.NCH, P).T))
        pc["dst_par"].append(np.ascontiguousarray(
            v_dpar.reshape(cfg.NCH, P).T))
        pc["src_par"].append(np.ascontiguousarray(
            v_spar.reshape(cfg.NCH, P).T))
    return pc


def _blockdiag_att(att, heads, hid, f):
    A = np.zeros((f, heads), dtype=np.float32)
    for h in range(heads):
        A[h * hid:(h + 1) * hid, h] = att[0, h]
    return A


def _ap(base, ap_list, off_extra=0):
    return bass.AP(tensor=base.tensor, offset=base.offset + off_extra,
                   ap=ap_list)


@with_exitstack
def _build_a(ctx, tc, cfg, t, repeat=1):
    for _rep in range(repeat):
        _build_a_once(tc, cfg, t)
        if _rep < repeat - 1:
            tc.strict_bb_all_engine_barrier()


@with_exitstack
def _build_a_once(ctx, tc, cfg, t):
    nc = tc.nc
    NCH_T, NCHL_T, NCHH_T, TPC = cfg.NCH_T, cfg.NCHL_T, cfg.NCHH_T, cfg.TPC
    WCOLS = cfg.F + 2 * cfg.H             # 144 matmul out cols
    TCOLS = cfg.F + cfg.H                 # 136 table1 used cols + s sep
    MCOLS = cfg.F + cfg.H                 # 136 message cols
    ROW1 = 256                            # table1 row elems (bf16)
    ROWS = 128                            # table_s row elems (bf16)
    W2COLS = cfg.OUT + 2

    consts = ctx.enter_context(tc.tile_pool(name="consts", bufs=1))
    wpack = consts.tile([P, WCOLS], F32)
    nc.sync.dma_start(out=wpack[:], in_=t["wpack"][:, :])
    w2pack = consts.tile([P, W2COLS], F32)
    nc.sync.dma_start(out=w2pack[:], in_=t["w2pack"][:, :])
    iota = consts.tile([P, P], F32)
    nc.sync.dma_start(out=iota[:], in_=t["iota"][:, :])
    ident = consts.tile([P, P], F32)
    nc.sync.dma_start(out=ident[:], in_=t["ident"][:, :])

    # ---------------- node phase ----------------
    NT = cfg.NTILES
    BLK = 8
    with ExitStack() as nctx:
        xpool = nctx.enter_context(tc.tile_pool(name="xt", bufs=2))
        npsum = nctx.enter_context(tc.tile_pool(name="npsum", bufs=2,
                                                space="PSUM"))
        nstage = nctx.enter_context(tc.tile_pool(name="nstage", bufs=3))
        for blk in range((NT + BLK - 1) // BLK):
            nt0 = blk * BLK
            nt1 = min(nt0 + BLK, NT)
            xt = xpool.tile([P, BLK * P], F32, tag="xt")
            nc.sync.dma_start(out=xt[:, 0:(nt1 - nt0) * P],
                              in_=t["xT"][:, nt0 * P:nt1 * P])
            for j in range(nt1 - nt0):
                nt = nt0 + j
                pt = npsum.tile([P, WCOLS], F32, tag="npt")
                nc.tensor.matmul(out=pt[:], lhsT=xt[:, j * P:(j + 1) * P],
                                 rhs=wpack[:], start=True, stop=True)
                s1 = nstage.tile([P, TCOLS], BF16, tag="s1")
                nc.scalar.activation(s1[:], pt[:, 0:TCOLS], AF.Copy)
                ss = nstage.tile([P, cfg.H], BF16, tag="ss")
                nc.scalar.activation(ss[:], pt[:, TCOLS:WCOLS], AF.Copy)
                nc.sync.dma_start(
                    out=t["table1"][nt * P:(nt + 1) * P, 0:TCOLS], in_=s1[:])
                nc.sync.dma_start(
                    out=t["table_s"][nt * P:(nt + 1) * P, 0:cfg.H],
                    in_=ss[:])

    # Tile does not track DRAM deps: fence table writes vs gathers.
    tc.strict_bb_all_engine_barrier()

    # ---------------- edge phase ----------------
    gpool = ctx.enter_context(tc.tile_pool(name="g", bufs=2))
    gspool = ctx.enter_context(tc.tile_pool(name="gs", bufs=2))
    ipool = ctx.enter_context(tc.tile_pool(name="idx", bufs=2))
    spool = ctx.enter_context(tc.tile_pool(name="sel", bufs=2))
    mpool = ctx.enter_context(tc.tile_pool(name="msg", bufs=2))
    lpool = ctx.enter_context(tc.tile_pool(name="logit", bufs=2))
    apsum = ctx.enter_context(tc.tile_pool(name="apsum", bufs=2,
                                           space="PSUM"))
    tpsum = ctx.enter_context(tc.tile_pool(name="tpsum", bufs=2,
                                           space="PSUM"))
    t2psum = ctx.enter_context(tc.tile_pool(name="t2psum", bufs=2,
                                            space="PSUM"))
    hpool = ctx.enter_context(tc.tile_pool(name="h1", bufs=2))

    tabs_pair = _ap(t["table_s"][:, :], [[256, cfg.N_PAD // 2], [1, 256]])
    tab_hi = t["table1"][cfg.SPLIT:cfg.N_PAD, :]

    for ti in range(TPC):
        c0 = ti * NCH_T
        # per-tile index tiles
        il = ipool.tile([P, NCHL_T * 8], I16, tag="il")
        nc.sync.dma_start(out=il[:], in_=t["srcw_lo"][
            :, ti * NCHL_T * 8:(ti + 1) * NCHL_T * 8])
        ih = ipool.tile([P, NCHH_T * 8], I16, tag="ih")
        nc.sync.dma_start(out=ih[:], in_=t["srcw_hi"][
            :, ti * NCHH_T * 8:(ti + 1) * NCHH_T * 8])
        idst = ipool.tile([P, NCH_T * 8], I16, tag="idst")
        nc.sync.dma_start(out=idst[:], in_=t["dstw"][
            :, ti * NCH_T * 8:(ti + 1) * NCH_T * 8])
        dloc = ipool.tile([P, NCH_T], F32, tag="dloc")
        nc.sync.dma_start(out=dloc[:], in_=t["dstloc_f"][:, c0:c0 + NCH_T])
        dpar = ipool.tile([P, NCH_T], F32, tag="dpar")
        nc.sync.dma_start(out=dpar[:], in_=t["dst_par"][:, c0:c0 + NCH_T])

        # gathers
        G = gpool.tile([P, NCH_T, ROW1], BF16, tag="G")
        nc.gpsimd.dma_gather(
            out_ap=G[:, 0:NCHL_T, :], in_ap=t["table1"][:, :],
            idxs_ap=il[:], num_idxs=NCHL_T * P, num_idxs_reg=NCHL_T * P,
            elem_size=ROW1, single_packet=False)
        nc.gpsimd.dma_gather(
            out_ap=G[:, NCHL_T:NCH_T, :], in_ap=tab_hi,
            idxs_ap=ih[:], num_idxs=NCHH_T * P, num_idxs_reg=NCHH_T * P,
            elem_size=ROW1, single_packet=False)
        Gs = gspool.tile([P, NCH_T, 256], BF16, tag="Gs")
        nc.gpsimd.dma_gather(
            out_ap=Gs[:], in_ap=tabs_pair, idxs_ap=idst[:],
            num_idxs=NCH_T * P, num_idxs_reg=NCH_T * P, elem_size=256, single_packet=False)

        # s1[dst] via parity select: s = odd + par*(even - odd)
        sd = spool.tile([P, NCH_T, cfg.H], F32, tag="sd")
        nc.vector.tensor_tensor(out=sd[:], in0=Gs[:, :, 0:cfg.H],
                                in1=Gs[:, :, ROWS:ROWS + cfg.H],
                                op=OP.subtract)
        sm = spool.tile([P, NCH_T, cfg.H], F32, tag="sm")
        nc.vector.tensor_tensor(
            out=sm[:], in0=sd[:],
            in1=_ap(dpar[:], [dpar[:].ap[0], [1, NCH_T], [0, cfg.H]]),
            op=OP.mult)
        so = spool.tile([P, NCH_T, cfg.H], F32, tag="so")
        nc.scalar.activation(so[:], Gs[:, :, ROWS:ROWS + cfg.H], AF.Copy)
        sp = spool.tile([P, NCH_T, cfg.H], F32, tag="sp")
        nc.vector.tensor_tensor(out=sp[:], in0=sm[:], in1=so[:], op=OP.add)

        # one-hot S
        S = spool.tile([P, NCH_T, P], BF16, tag="S")
        nc.vector.tensor_tensor(
            out=S[:],
            in0=_ap(iota[:], [iota[:].ap[0], [0, NCH_T], [1, P]]),
            in1=_ap(dloc[:], [dloc[:].ap[0], [1, NCH_T], [0, P]]),
            op=OP.is_equal)

        # logits -> ex (bf16)
        dcp = lpool.tile([P, NCH_T, cfg.H], F32, tag="dcp")
        nc.scalar.activation(dcp[:], G[:, :, cfg.F:cfg.F + cfg.H], AF.Copy)
        u = lpool.tile([P, NCH_T, cfg.H], F32, tag="u")
        nc.vector.tensor_tensor(out=u[:], in0=sp[:], in1=dcp[:], op=OP.add)
        a = lpool.tile([P, NCH_T, cfg.H], F32, tag="a")
        nc.vector.scalar_tensor_tensor(out=a[:], in0=u[:], scalar=cfg.NEG,
                                       in1=u[:], op0=OP.mult, op1=OP.max)
        ex = lpool.tile([P, NCH_T, cfg.H], BF16, tag="ex")
        nc.scalar.activation(ex[:], a[:], AF.Exp)

        # Msg = [feat * ex | ex]
        M = mpool.tile([P, NCH_T, MCOLS], BF16, tag="M")
        nc.scalar.activation(M[:, :, cfg.F:MCOLS], ex[:], AF.Copy)
        nc.vector.tensor_tensor(
            out=_ap(M[:], [M[:].ap[0], [MCOLS, NCH_T], [cfg.HID, cfg.H],
                           [1, cfg.HID]]),
            in0=_ap(G[:], [G[:].ap[0], [ROW1, NCH_T], [cfg.HID, cfg.H],
                           [1, cfg.HID]]),
            in1=_ap(ex[:], [ex[:].ap[0], [cfg.H, NCH_T], [1, cfg.H],
                            [0, cfg.HID]]),
            op=OP.mult)

        # aggregate
        agg = apsum.tile([P, MCOLS], F32, tag="agg")
        for k in range(NCH_T):
            nc.tensor.matmul(out=agg[:], lhsT=S[:, k, :], rhs=M[:, k, :],
                             start=(k == 0), stop=(k == NCH_T - 1))

        # normalize + elu + feat2/d2/s2 slab
        den = hpool.tile([P, cfg.H], F32, tag="den")
        nc.vector.tensor_scalar_add(den[:], agg[:, cfg.F:MCOLS], 1e-20)
        rcp = hpool.tile([P, cfg.H], F32, tag="rcp")
        nc.vector.reciprocal(rcp[:], den[:])
        h1 = hpool.tile([P, cfg.F], F32, tag="h1")
        nc.vector.tensor_tensor(
            out=_ap(h1[:], [h1[:].ap[0], [cfg.HID, cfg.H], [1, cfg.HID]]),
            in0=_ap(agg[:], [agg[:].ap[0], [cfg.HID, cfg.H], [1, cfg.HID]]),
            in1=_ap(rcp[:], [rcp[:].ap[0], [1, cfg.H], [0, cfg.HID]]),
            op=OP.mult)
        neg = hpool.tile([P, cfg.F], F32, tag="neg")
        nc.vector.tensor_scalar_min(neg[:], h1[:], 0.0)
        pos = hpool.tile([P, cfg.F], F32, tag="pos")
        nc.vector.tensor_scalar_max(pos[:], h1[:], 0.0)
        een = hpool.tile([P, cfg.F], F32, tag="een")
        nc.scalar.activation(een[:], neg[:], AF.Exp)
        elu = hpool.tile([P, cfg.F], F32, tag="elu")
        nc.vector.scalar_tensor_tensor(out=elu[:], in0=een[:], scalar=-1.0,
                                       in1=pos[:], op0=OP.add, op1=OP.add)
        eT_ps = tpsum.tile([P, P], F32, tag="eT")
        nc.tensor.transpose(out=eT_ps[:], in_=elu[:], identity=ident[:])
        eT = hpool.tile([P, P], F32, tag="eTs")
        nc.scalar.activation(eT[:], eT_ps[:], AF.Copy)
        t2 = t2psum.tile([P, W2COLS], F32, tag="t2")
        nc.tensor.matmul(out=t2[:], lhsT=eT[:], rhs=w2pack[:],
                         start=True, stop=True)
        t2s = hpool.tile([P, W2COLS], F32, tag="t2s")
        nc.scalar.activation(t2s[:], t2[:], AF.Copy)
        nc.sync.dma_start(out=t["table2slab"][ti * P:(ti + 1) * P, :],
                          in_=t2s[:])


@with_exitstack
def _build_b(ctx, tc, cfg, t, repeat=1):
    for _rep in range(repeat):
        _build_b_once(tc, cfg, t)
        if _rep < repeat - 1:
            tc.strict_bb_all_engine_barrier()


@with_exitstack
def _build_b_once(ctx, tc, cfg, t):
    nc = tc.nc
    NCH_T, TPC = cfg.NCH_T, cfg.TPC
    UC = cfg.OUT + 2                    # 18 used cols in table2
    MC = cfg.OUT + 1                    # 17 message cols

    consts = ctx.enter_context(tc.tile_pool(name="consts", bufs=1))
    iota = consts.tile([P, P], F32)
    nc.sync.dma_start(out=iota[:], in_=t["iota"][:, :])

    ipool = ctx.enter_context(tc.tile_pool(name="idx", bufs=2))
    gpool = ctx.enter_context(tc.tile_pool(name="g2", bufs=2))
    spool = ctx.enter_context(tc.tile_pool(name="s2", bufs=2))
    lpool = ctx.enter_context(tc.tile_pool(name="l2", bufs=2))
    mpool = ctx.enter_context(tc.tile_pool(name="m2", bufs=2))
    apsum = ctx.enter_context(tc.tile_pool(name="aps2", bufs=2,
                                           space="PSUM"))
    opool = ctx.enter_context(tc.tile_pool(name="o", bufs=3))

    tab_pair = _ap(t["table2"][:, :], [[256, cfg.N_PAD // 2], [1, 256]])

    for ti in range(TPC):
        c0 = ti * NCH_T
        isrc = ipool.tile([P, NCH_T * 8], I16, tag="isrc")
        nc.sync.dma_start(out=isrc[:], in_=t["srcw"][
            :, ti * NCH_T * 8:(ti + 1) * NCH_T * 8])
        idst = ipool.tile([P, NCH_T * 8], I16, tag="idst")
        nc.sync.dma_start(out=idst[:], in_=t["dstw"][
            :, ti * NCH_T * 8:(ti + 1) * NCH_T * 8])
        dloc = ipool.tile([P, NCH_T], F32, tag="dloc")
        nc.sync.dma_start(out=dloc[:], in_=t["dstloc_f"][:, c0:c0 + NCH_T])
        dpar = ipool.tile([P, NCH_T], F32, tag="dpar")
        nc.sync.dma_start(out=dpar[:], in_=t["dst_par"][:, c0:c0 + NCH_T])
        spar = ipool.tile([P, NCH_T], F32, tag="spar")
        nc.sync.dma_start(out=spar[:], in_=t["src_par"][:, c0:c0 + NCH_T])

        G2 = gpool.tile([P, NCH_T, 256], BF16, tag="G2")
        nc.gpsimd.dma_gather(
            out_ap=G2[:], in_ap=tab_pair, idxs_ap=isrc[:],
            num_idxs=NCH_T * P, num_idxs_reg=NCH_T * P, elem_size=256, single_packet=False)
        Gd2 = gpool.tile([P, NCH_T, 256], BF16, tag="Gd2")
        nc.gpsimd.dma_gather(
            out_ap=Gd2[:], in_ap=tab_pair, idxs_ap=idst[:],
            num_idxs=NCH_T * P, num_idxs_reg=NCH_T * P, elem_size=256, single_packet=False)

        # parity selects: x = odd + par*(even - odd)
        Rd = spool.tile([P, NCH_T, UC], F32, tag="Rd")
        nc.vector.tensor_tensor(out=Rd[:], in0=G2[:, :, 0:UC],
                                in1=G2[:, :, 128:128 + UC], op=OP.subtract)
        Rm = spool.tile([P, NCH_T, UC], F32, tag="Rm")
        nc.vector.tensor_tensor(
            out=Rm[:], in0=Rd[:],
            in1=_ap(spar[:], [spar[:].ap[0], [1, NCH_T], [0, UC]]),
            op=OP.mult)
        Ro = spool.tile([P, NCH_T, UC], F32, tag="Ro")
        nc.scalar.activation(Ro[:], G2[:, :, 128:128 + UC], AF.Copy)
        R = spool.tile([P, NCH_T, UC], F32, tag="R")
        nc.vector.tensor_tensor(out=R[:], in0=Rm[:], in1=Ro[:], op=OP.add)

        s2d = spool.tile([P, NCH_T, 1], F32, tag="s2d")
        nc.vector.tensor_tensor(out=s2d[:], in0=Gd2[:, :, MC:MC + 1],
                                in1=Gd2[:, :, 128 + MC:128 + MC + 1],
                                op=OP.subtract)
        s2m = spool.tile([P, NCH_T, 1], F32, tag="s2m")
        nc.vector.tensor_tensor(
            out=s2m[:], in0=s2d[:],
            in1=_ap(dpar[:], [dpar[:].ap[0], [1, NCH_T], [0, 1]]),
            op=OP.mult)
        s2o = spool.tile([P, NCH_T, 1], F32, tag="s2o")
        nc.scalar.activation(s2o[:], Gd2[:, :, 128 + MC:128 + MC + 1],
                             AF.Copy)
        s2 = spool.tile([P, NCH_T, 1], F32, tag="s2")
        nc.vector.tensor_tensor(out=s2[:], in0=s2m[:], in1=s2o[:], op=OP.add)

        S = spool.tile([P, NCH_T, P], BF16, tag="S")
        nc.vector.tensor_tensor(
            out=S[:],
            in0=_ap(iota[:], [iota[:].ap[0], [0, NCH_T], [1, P]]),
            in1=_ap(dloc[:], [dloc[:].ap[0], [1, NCH_T], [0, P]]),
            op=OP.is_equal)

        u = lpool.tile([P, NCH_T, 1], F32, tag="u2")
        nc.vector.tensor_tensor(out=u[:], in0=s2[:],
                                in1=R[:, :, cfg.OUT:MC], op=OP.add)
        a = lpool.tile([P, NCH_T, 1], F32, tag="a2")
        nc.vector.scalar_tensor_tensor(out=a[:], in0=u[:], scalar=cfg.NEG,
                                       in1=u[:], op0=OP.mult, op1=OP.max)
        ex = lpool.tile([P, NCH_T, 1], F32, tag="ex2")
        nc.scalar.activation(ex[:], a[:], AF.Exp)

        M = mpool.tile([P, NCH_T, MC], BF16, tag="M2")
        nc.scalar.activation(M[:, :, cfg.OUT:MC], ex[:], AF.Copy)
        nc.vector.tensor_tensor(
            out=M[:, :, 0:cfg.OUT],
            in0=R[:, :, 0:cfg.OUT],
            in1=_ap(ex[:], [ex[:].ap[0], [1, NCH_T], [0, cfg.OUT]]),
            op=OP.mult)

        agg = apsum.tile([P, MC], F32, tag="agg2")
        for k in range(NCH_T):
            nc.tensor.matmul(out=agg[:], lhsT=S[:, k, :], rhs=M[:, k, :],
                             start=(k == 0), stop=(k == NCH_T - 1))

        den = opool.tile([P, 1], F32, tag="den")
        nc.vector.tensor_scalar_add(den[:], agg[:, cfg.OUT:MC], 1e-20)
        rcp = opool.tile([P, 1], F32, tag="rcp")
        nc.vector.reciprocal(rcp[:], den[:])
        h2 = opool.tile([P, cfg.OUT], F32, tag="h2")
        nc.vector.tensor_tensor(
            out=h2[:], in0=agg[:, 0:cfg.OUT],
            in1=_ap(rcp[:], [rcp[:].ap[0], [0, cfg.OUT]]), op=OP.mult)
        m = opool.tile([P, 1], F32, tag="m")
        nc.vector.tensor_reduce(out=m[:], in_=h2[:],
                                axis=mybir.AxisListType.X, op=OP.max)
        tm = opool.tile([P, cfg.OUT], F32, tag="tm")
        nc.vector.tensor_tensor(
            out=tm[:], in0=h2[:],
            in1=_ap(m[:], [m[:].ap[0], [0, cfg.OUT]]), op=OP.subtract)
        pe = opool.tile([P, cfg.OUT], F32, tag="pe")
        ssum = opool.tile([P, 1], F32, tag="ss")
        nc.scalar.activation(pe[:], tm[:], AF.Exp, accum_out=ssum[:])
        ln = opool.tile([P, 1], F32, tag="ln")
        nc.scalar.activation(ln[:], ssum[:], AF.Ln)
        res = opool.tile([P, cfg.OUT], F32, tag="res")
        nc.vector.tensor_tensor(
            out=res[:], in0=tm[:],
            in1=_ap(ln[:], [ln[:].ap[0], [0, cfg.OUT]]), op=OP.subtract)
        nc.sync.dma_start(out=t["outp"][ti * P:(ti + 1) * P, :], in_=res[:])


def _decl_a(nc, cfg):
    t = {}
    WCOLS = cfg.F + 2 * cfg.H
    W2COLS = cfg.OUT + 2

    def inp(name, shape, dt):
        t[name] = nc.dram_tensor(name, shape, dt, kind="ExternalInput").ap()

    inp("xT", [P, cfg.N_PAD], F32)
    inp("wpack", [P, WCOLS], F32)
    inp("w2pack", [P, W2COLS], F32)
    inp("iota", [P, P], F32)
    inp("ident", [P, P], F32)
    inp("srcw_lo", [P, cfg.TPC * cfg.NCHL_T * 8], I16)
    inp("srcw_hi", [P, cfg.TPC * cfg.NCHH_T * 8], I16)
    inp("dstw", [P, cfg.NCH * 8], I16)
    inp("dstloc_f", [P, cfg.NCH], F32)
    inp("dst_par", [P, cfg.NCH], F32)
    t["table1"] = nc.dram_tensor("table1", [cfg.N_PAD, 256], BF16,
                                 kind="Internal").ap()
    t["table_s"] = nc.dram_tensor("table_s", [cfg.N_PAD, 128], BF16,
                                  kind="Internal").ap()
    t["table2slab"] = nc.dram_tensor("table2slab", [cfg.NPC_PAD, W2COLS],
                                     F32, kind="ExternalOutput").ap()
    return t


def _decl_b(nc, cfg):
    t = {}

    def inp(name, shape, dt):
        t[name] = nc.dram_tensor(name, shape, dt, kind="ExternalInput").ap()

    inp("table2", [cfg.N_PAD, 128], BF16)
    inp("srcw", [P, cfg.NCH * 8], I16)
    inp("dstw", [P, cfg.NCH * 8], I16)
    inp("dstloc_f", [P, cfg.NCH], F32)
    inp("dst_par", [P, cfg.NCH], F32)
    inp("src_par", [P, cfg.NCH], F32)
    inp("iota", [P, P], F32)
    t["outp"] = nc.dram_tensor("outp", [cfg.NPC_PAD, cfg.OUT], F32,
                               kind="ExternalOutput").ap()
    return t


def _compile(build_fn, decl_fn, cfg, repeat=1):
    nc = bacc.Bacc("TRN2", target_bir_lowering=False, debug=False,
                   enable_asserts=False, num_devices=cfg.NCORES)
    t = decl_fn(nc, cfg)
    with tile.TileContext(nc) as tc:
        build_fn(tc, cfg, t, repeat=repeat)
    nc.compile()
    return nc


def _host_prep_weights(cfg, W1, att_src1, att_dst1, W2, att_src2, att_dst2):
    A_d1 = _blockdiag_att(np.asarray(att_dst1, np.float32), cfg.H, cfg.HID,
                          cfg.F)
    A_s1 = _blockdiag_att(np.asarray(att_src1, np.float32), cfg.H, cfg.HID,
                          cfg.F)
    W1T = np.asarray(W1, np.float32).T.copy()
    wpack = np.concatenate([W1T, W1T @ A_d1, W1T @ A_s1], axis=1)
    W2T = np.asarray(W2, np.float32).T.copy()
    a_d2 = np.asarray(att_dst2, np.float32).reshape(cfg.OUT, 1)
    a_s2 = np.asarray(att_src2, np.float32).reshape(cfg.OUT, 1)
    w2pack = np.concatenate([W2T, W2T @ a_d2, W2T @ a_s2], axis=1)
    return (np.ascontiguousarray(wpack, np.float32),
            np.ascontiguousarray(w2pack, np.float32))


_CACHE = {}


def _get_kernels(cfg):
    key = (cfg.N, cfg.E, cfg.NCORES, cfg.NCH_T, cfg.NCHL_T)
    if key not in _CACHE:
        nca = _compile(_build_a, _decl_a, cfg)
        ncb = _compile(_build_b, _decl_b, cfg)
        _CACHE[key] = (nca, ncb)
    return _CACHE[key]


def run(cfg, inputs, runner=None):
    x = np.asarray(inputs["x"], np.float32)
    edge_index = np.asarray(inputs["edge_index"], np.int64)
    pc = _prep_graph(cfg, edge_index)
    wpack, w2pack = _host_prep_weights(
        cfg, inputs["W1"], inputs["att_src1"], inputs["att_dst1"],
        inputs["W2"], inputs["att_src2"], inputs["att_dst2"])

    xT = np.zeros((P, cfg.N_PAD), np.float32)
    xT[:, :cfg.N] = x.T
    iota = np.tile(np.arange(P, dtype=np.float32), (P, 1))
    ident = np.eye(P, dtype=np.float32)

    nca, ncb = _get_kernels(cfg)

    if runner is None:
        def runner(nc, in_maps):
            r = bass_utils.run_bass_kernel_spmd(
                nc, in_maps, core_ids=list(range(cfg.NCORES)))
            return r.results

    in_maps_a = []
    for c in range(cfg.NCORES):
        in_maps_a.append(dict(
            xT=xT, wpack=wpack, w2pack=w2pack, iota=iota, ident=ident,
            srcw_lo=pc["srcw_lo"][c], srcw_hi=pc["srcw_hi"][c],
            dstw=pc["dstw"][c], dstloc_f=pc["dstloc_f"][c],
            dst_par=pc["dst_par"][c]))
    res_a = runner(nca, in_maps_a)

    table2 = np.zeros((cfg.N_PAD, 128), BF)
    for c in range(cfg.NCORES):
        slab = np.asarray(res_a[c]["table2slab"], np.float32)
        table2[c * cfg.NPC:(c + 1) * cfg.NPC, 0:cfg.OUT + 2] = \
            slab[:cfg.NPC].astype(BF)

    in_maps_b = []
    for c in range(cfg.NCORES):
        in_maps_b.append(dict(
            table2=table2, srcw=pc["srcw"][c], dstw=pc["dstw"][c],
            dstloc_f=pc["dstloc_f"][c], dst_par=pc["dst_par"][c],
            src_par=pc["src_par"][c], iota=iota))
    res_b = runner(ncb, in_maps_b)

    out = np.zeros((cfg.N, cfg.OUT), np.float32)
    for c in range(cfg.NCORES):
        out[c * cfg.NPC:(c + 1) * cfg.NPC] = \
            np.asarray(res_b[c]["outp"], np.float32)[:cfg.NPC]
    return out


def kernel(**inputs):
    cfg = Cfg(N=50000, E=1600000, ncores=8)
    return run(cfg, inputs)



# revision 2
# speedup vs baseline: 2.4840x; 2.4840x over previous
"""
2-layer GAT on Trainium2 (8 NeuronCores, SPMD via bass/Tile) — v2.

Sharding: destination nodes block-sharded across 8 cores (6250 each).
All per-edge work runs on the core owning the edge's dst.  Layer-0
node-level compute (h = x @ W1pack, bf16) is replicated on every core.
Two kernels (A: layer 1, B: layer 2) with a host gather of per-core
node tables in between.

Key performance structure (vs v1 baseline, 8.29 ms -> target ~1.3 ms):
  - 4 SWDGE queues (num_swdge_queues=4): dma_gather descriptor
    generation parallelises over all 4 Q7 cpu pairs (2.33 ns/idx vs
    7.92 measured).
  - ONE gather index per edge per layer: the dst-side attention scalar
    (s1/s2) is no longer gathered per edge.  Instead each dst tile
    fetches its 128 nodes' scalar with two tiny 128-idx gathers
    (lo/hi of the node id, masked combine), then broadcasts to edge
    slots with a per-chunk PE matmul  lhsT=ST (transposed one-hot).
  - One-hot matrices S (slot->dstlocal, aggregation lhsT) and ST
    (broadcast lhsT) are precomputed on host as fp8 and streamed.
  - Messages M are fp8 (keeps the big DVE mult in 1x mode: no
    GPSIMD/SWDGE port-pair lock), aggregation matmul fp8 x fp8.
  - Node phase in bf16 (fp32 matmul is 4x slower on PE).
"""

import os
import sys

import numpy as np
import ml_dtypes

for _p in ("/opt/trn_rl_repo",):
    if os.path.isdir(_p) and _p not in sys.path:
        sys.path.insert(0, _p)

import concourse.bass as bass
import concourse.bacc as bacc
import concourse.tile as tile
from concourse import mybir
from concourse import bass_utils
from concourse._compat import with_exitstack
from contextlib import ExitStack

F32 = mybir.dt.float32
BF16 = mybir.dt.bfloat16
FP8 = mybir.dt.float8e4
I32 = mybir.dt.int32
I16 = mybir.dt.int16
AF = mybir.ActivationFunctionType
OP = mybir.AluOpType
P = 128
BF = ml_dtypes.bfloat16
F8 = ml_dtypes.float8_e4m3
NQ = 4                     # SWDGE queues


class Cfg:
    def __init__(self, N, E, ncores, split=32768, neg=0.2, in_ch=128,
                 f=128, heads=8, hid=16, out=16):
        self.N = N
        self.E = E
        self.NCORES = ncores
        self.SPLIT = split
        self.NEG = neg
        self.IN = in_ch
        self.F = f
        self.H = heads
        self.HID = hid
        self.OUT = out
        assert N % ncores == 0
        self.NPC = N // ncores
        self.TPC = (self.NPC + P - 1) // P
        self.NPC_PAD = self.TPC * P
        self.NTILES = ncores * self.TPC
        self.N_PAD = self.NTILES * P
        # filled by _prep_graph
        self.NCHL_T = None   # lo chunks per tile (layer A)
        self.NCHH_T = None   # hi chunks per tile (layer A)
        self.NCH_T = None
        self.NCE_T = None    # even-src chunks per tile (layer B)
        self.NCO_T = None    # odd-src chunks per tile (layer B)
        self.NCB_T = None


def _wrap16(vals):
    """[n] slot-ordered int idx -> [128, n//16] int16 wrapped layout."""
    n = vals.shape[0]
    assert n % 16 == 0
    w = vals.reshape(-1, 16).T.astype(np.int16)
    return np.ascontiguousarray(np.tile(w, (8, 1)))


def _slot_fill(src_vals, dloc_vals, pos, nch_grp, grp_off, t, NCH_T, v_idx,
               s_rows, s_cols, st_rows, st_cols):
    """Record slot assignments for one (tile, group) run of edges."""
    chunk = t * NCH_T + grp_off + pos // P
    part = pos % P
    v_idx[...] = src_vals
    s_rows.append(part)
    s_cols.append(chunk * P + dloc_vals)
    st_rows.append(dloc_vals)
    st_cols.append(chunk * P + part)


def _prep_graph(cfg, edge_index):
    N, NPC, TPC, SPL = cfg.N, cfg.NPC, cfg.TPC, cfg.SPLIT
    src = np.concatenate([edge_index[0], np.arange(N, dtype=np.int64)])
    dst = np.concatenate([edge_index[1], np.arange(N, dtype=np.int64)])
    core = dst // NPC
    ld = dst - core * NPC
    tile_id = ld // P
    dloc = ld % P

    # ---- layer A grouping: (core, tile, hi(src), src) ----
    hi = (src >= SPL).astype(np.int64)
    orderA = np.lexsort((src, hi, tile_id, core))
    keyA = (core * TPC + tile_id) * 2 + hi
    cntA = np.bincount(keyA, minlength=cfg.NCORES * TPC * 2)
    cnt_lo = cntA[0::2].reshape(cfg.NCORES, TPC)
    cnt_hi = cntA[1::2].reshape(cfg.NCORES, TPC)
    cfg.NCHL_T = max(1, int(np.max((cnt_lo + P - 1) // P)))
    cfg.NCHH_T = max(1, int(np.max((cnt_hi + P - 1) // P)))
    cfg.NCH_T = cfg.NCHL_T + cfg.NCHH_T

    # ---- layer B grouping: (core, tile, parity(src), src) ----
    par = (src & 1).astype(np.int64)
    orderB = np.lexsort((src, par, tile_id, core))
    keyB = (core * TPC + tile_id) * 2 + par
    cntB = np.bincount(keyB, minlength=cfg.NCORES * TPC * 2)
    cnt_ev = cntB[0::2].reshape(cfg.NCORES, TPC)
    cnt_od = cntB[1::2].reshape(cfg.NCORES, TPC)
    cfg.NCE_T = max(1, int(np.max((cnt_ev + P - 1) // P)))
    cfg.NCO_T = max(1, int(np.max((cnt_od + P - 1) // P)))
    cfg.NCB_T = cfg.NCE_T + cfg.NCO_T

    startsA = np.concatenate([[0], np.cumsum(cntA)])
    startsB = np.concatenate([[0], np.cumsum(cntB)])
    ONE = np.uint8(0x38)  # 1.0 in float8_e4m3

    pc = dict(srcw_lo=[], srcw_hi=[], srcw_b=[], S_A=[], ST_A=[],
              S_B=[], ST_B=[], s1idx_lo=[], s1idx_hi=[], m_lo=[], m_hi=[])
    sA, dA = src[orderA], dloc[orderA]
    sB, dB = src[orderB], dloc[orderB]
    for c in range(cfg.NCORES):
        v_lo = np.zeros(TPC * cfg.NCHL_T * P, np.int64)
        v_hi = np.zeros(TPC * cfg.NCHH_T * P, np.int64)
        v_b = np.zeros(TPC * cfg.NCB_T * P, np.int64)
        SA = np.zeros((P, TPC * cfg.NCH_T * P), np.uint8)
        STA = np.zeros((P, TPC * cfg.NCH_T * P), np.uint8)
        SB = np.zeros((P, TPC * cfg.NCB_T * P), np.uint8)
        STB = np.zeros((P, TPC * cfg.NCB_T * P), np.uint8)
        for t in range(TPC):
            for g in (0, 1):
                # layer A
                k = (c * TPC + t) * 2 + g
                n = int(cntA[k])
                if n:
                    sl = slice(startsA[k], startsA[k] + n)
                    e_src, e_dl = sA[sl], dA[sl]
                    pos = np.arange(n)
                    if g == 0:
                        v_lo[t * cfg.NCHL_T * P + pos] = e_src
                        chunk = t * cfg.NCH_T + pos // P
                    else:
                        v_hi[t * cfg.NCHH_T * P + pos] = e_src - SPL
                        chunk = t * cfg.NCH_T + cfg.NCHL_T + pos // P
                    part = pos % P
                    SA[part, chunk * P + e_dl] = ONE
                    STA[e_dl, chunk * P + part] = ONE
                # layer B
                n = int(cntB[k])
                if n:
                    sl = slice(startsB[k], startsB[k] + n)
                    e_src, e_dl = sB[sl], dB[sl]
                    pos = np.arange(n)
                    if g == 0:
                        chunk = t * cfg.NCB_T + pos // P
                    else:
                        chunk = t * cfg.NCB_T + cfg.NCE_T + pos // P
                    v_b[chunk * P + pos % P] = e_src >> 1
                    part = pos % P
                    SB[part, chunk * P + e_dl] = ONE
                    STB[e_dl, chunk * P + part] = ONE
        pc["srcw_lo"].append(_wrap16(v_lo))
        pc["srcw_hi"].append(_wrap16(v_hi))
        pc["srcw_b"].append(_wrap16(v_b))
        pc["S_A"].append(SA.view(F8))
        pc["ST_A"].append(STA.view(F8))
        pc["S_B"].append(SB.view(F8))
        pc["ST_B"].append(STB.view(F8))
        # dst-tile node-id gathers for s1/s2 (lo/hi + mask combine)
        nodes = c * NPC + np.arange(cfg.NPC_PAD, dtype=np.int64)
        lo_sel = nodes < SPL
        idx_lo = np.where(lo_sel, nodes, 0)
        idx_hi = np.where(lo_sel, 0, nodes - SPL)
        pc["s1idx_lo"].append(_wrap16(idx_lo))
        pc["s1idx_hi"].append(_wrap16(idx_hi))
        m = lo_sel.astype(np.float32).reshape(TPC, P).T   # [128, TPC]
        pc["m_lo"].append(np.ascontiguousarray(m.astype(BF)))
        pc["m_hi"].append(np.ascontiguousarray((1.0 - m).astype(BF)))
    return pc


def _blockdiag_att(att, heads, hid, f):
    A = np.zeros((f, heads), dtype=np.float32)
    for h in range(heads):
        A[h * hid:(h + 1) * hid, h] = att[0, h]
    return A


def _ap(base, ap_list, off_extra=0):
    return bass.AP(tensor=base.tensor, offset=base.offset + off_extra,
                   ap=ap_list)


@with_exitstack
def _build_a(ctx, tc, cfg, t):
    nc = tc.nc
    NCHL_T, NCHH_T, NCH_T, TPC = cfg.NCHL_T, cfg.NCHH_T, cfg.NCH_T, cfg.TPC
    WCOLS = cfg.F + 2 * cfg.H             # 144 matmul out cols
    TCOLS = cfg.F + cfg.H                 # 136 table1 used cols
    MCOLS = cfg.F + cfg.H                 # 136 message cols
    ROW1 = 256                            # table1 row elems (bf16, 512B)
    ROWS = 128                            # table_s row elems (bf16, 256B)
    W2COLS = cfg.OUT + 2

    consts = ctx.enter_context(tc.tile_pool(name="consts", bufs=1))
    wpack = consts.tile([P, WCOLS], BF16)
    nc.sync.dma_start(out=wpack[:], in_=t["wpack"][:, :])
    w2pack = consts.tile([P, W2COLS], BF16)
    nc.sync.dma_start(out=w2pack[:], in_=t["w2pack"][:, :])
    ident = consts.tile([P, P], F32)
    nc.sync.dma_start(out=ident[:], in_=t["ident"][:, :])
    mlo = consts.tile([P, TPC], BF16)
    nc.sync.dma_start(out=mlo[:], in_=t["m_lo"][:, :])
    mhi = consts.tile([P, TPC], BF16)
    nc.sync.dma_start(out=mhi[:], in_=t["m_hi"][:, :])
    # all idx tiles loaded once
    il_all = consts.tile([P, TPC * NCHL_T * 8], I16)
    nc.sync.dma_start(out=il_all[:], in_=t["srcw_lo"][:, :])
    ih_all = consts.tile([P, TPC * NCHH_T * 8], I16)
    nc.sync.dma_start(out=ih_all[:], in_=t["srcw_hi"][:, :])
    isl_all = consts.tile([P, TPC * 8], I16)
    nc.sync.dma_start(out=isl_all[:], in_=t["s1idx_lo"][:, :])
    ish_all = consts.tile([P, TPC * 8], I16)
    nc.sync.dma_start(out=ish_all[:], in_=t["s1idx_hi"][:, :])

    # ---------------- node phase (bf16) ----------------
    NT = cfg.NTILES
    BLK = 8
    with ExitStack() as nctx:
        xpool = nctx.enter_context(tc.tile_pool(name="xt", bufs=2))
        npsum = nctx.enter_context(tc.tile_pool(name="npsum", bufs=4,
                                                space="PSUM"))
        nstage = nctx.enter_context(tc.tile_pool(name="nstage", bufs=3))
        for blk in range((NT + BLK - 1) // BLK):
            nt0 = blk * BLK
            nt1 = min(nt0 + BLK, NT)
            xt = xpool.tile([P, BLK * P], BF16, tag="xt")
            nc.sync.dma_start(out=xt[:, 0:(nt1 - nt0) * P],
                              in_=t["xT"][:, nt0 * P:nt1 * P])
            for j in range(nt1 - nt0):
                nt = nt0 + j
                pt = npsum.tile([P, WCOLS], F32, tag="npt")
                nc.tensor.matmul(out=pt[:], lhsT=xt[:, j * P:(j + 1) * P],
                                 rhs=wpack[:], start=True, stop=True)
                s1 = nstage.tile([P, TCOLS], BF16, tag="s1")
                nc.scalar.activation(s1[:], pt[:, 0:TCOLS], AF.Copy)
                ss = nstage.tile([P, cfg.H], BF16, tag="ss")
                nc.scalar.activation(ss[:], pt[:, TCOLS:WCOLS], AF.Copy)
                nc.sync.dma_start(
                    out=t["table1"][nt * P:(nt + 1) * P, 0:TCOLS], in_=s1[:])
                nc.sync.dma_start(
                    out=t["table_s"][nt * P:(nt + 1) * P, 0:cfg.H],
                    in_=ss[:])

    # Tile does not track DRAM deps: fence table writes vs gathers.
    tc.strict_bb_all_engine_barrier()

    # ---------------- edge phase ----------------
    gpool = ctx.enter_context(tc.tile_pool(name="g", bufs=2))
    spool = ctx.enter_context(tc.tile_pool(name="soh", bufs=3))
    slpool = ctx.enter_context(tc.tile_pool(name="sloc", bufs=2))
    lpool = ctx.enter_context(tc.tile_pool(name="logit", bufs=2))
    mpool = ctx.enter_context(tc.tile_pool(name="msg", bufs=2))
    lpsum = ctx.enter_context(tc.tile_pool(name="lpsum", bufs=2,
                                           space="PSUM"))
    apsum = ctx.enter_context(tc.tile_pool(name="apsum", bufs=2,
                                           space="PSUM"))
    tpsum = ctx.enter_context(tc.tile_pool(name="tpsum", bufs=2,
                                           space="PSUM"))
    t2psum = ctx.enter_context(tc.tile_pool(name="t2psum", bufs=2,
                                            space="PSUM"))
    hpool = ctx.enter_context(tc.tile_pool(name="h1", bufs=2))

    tab_hi = t["table1"][cfg.SPLIT:cfg.N_PAD, :]
    tabs_hi = t["table_s"][cfg.SPLIT:cfg.N_PAD, :]
    q = [0]

    def nextq():
        r = q[0] % NQ
        q[0] += 1
        return r

    for ti in range(TPC):
        # gathers: feat+d1 rows by src (lo/hi), s1 rows for own nodes
        G = gpool.tile([P, NCH_T, ROW1], BF16, tag="G")
        nc.gpsimd.dma_gather(
            out_ap=G[:, 0:NCHL_T, :], in_ap=t["table1"][:, :],
            idxs_ap=il_all[:, ti * NCHL_T * 8:(ti + 1) * NCHL_T * 8],
            num_idxs=NCHL_T * P, num_idxs_reg=NCHL_T * P,
            elem_size=ROW1, single_packet=False, queue_num=nextq())
        nc.gpsimd.dma_gather(
            out_ap=G[:, NCHL_T:NCH_T, :], in_ap=tab_hi,
            idxs_ap=ih_all[:, ti * NCHH_T * 8:(ti + 1) * NCHH_T * 8],
            num_idxs=NCHH_T * P, num_idxs_reg=NCHH_T * P,
            elem_size=ROW1, single_packet=False, queue_num=nextq())
        s1A = slpool.tile([P, 1, ROWS], BF16, tag="s1A")
        nc.gpsimd.dma_gather(
            out_ap=s1A[:], in_ap=t["table_s"][:, :],
            idxs_ap=isl_all[:, ti * 8:(ti + 1) * 8],
            num_idxs=P, num_idxs_reg=P, elem_size=ROWS,
            single_packet=False, queue_num=nextq())
        s1B = slpool.tile([P, 1, ROWS], BF16, tag="s1B")
        nc.gpsimd.dma_gather(
            out_ap=s1B[:], in_ap=tabs_hi,
            idxs_ap=ish_all[:, ti * 8:(ti + 1) * 8],
            num_idxs=P, num_idxs_reg=P, elem_size=ROWS,
            single_packet=False, queue_num=nextq())

        # one-hot streams
        S = spool.tile([P, NCH_T * P], FP8, tag="S")
        nc.sync.dma_start(out=S[:], in_=t["S_A"][
            :, ti * NCH_T * P:(ti + 1) * NCH_T * P])
        ST = spool.tile([P, NCH_T * P], FP8, tag="ST")
        nc.sync.dma_start(out=ST[:], in_=t["ST_A"][
            :, ti * NCH_T * P:(ti + 1) * NCH_T * P])

        # s1loc[j, :] = s1 of the tile's j-th node (lo/hi masked)
        sA = slpool.tile([P, cfg.H], BF16, tag="sA")
        nc.vector.tensor_tensor(
            out=sA[:], in0=s1A[:, 0, 0:cfg.H],
            in1=_ap(mlo[:], [mlo[:].ap[0], [0, cfg.H]], off_extra=ti),
            op=OP.mult)
        sB = slpool.tile([P, cfg.H], BF16, tag="sB")
        nc.vector.tensor_tensor(
            out=sB[:], in0=s1B[:, 0, 0:cfg.H],
            in1=_ap(mhi[:], [mhi[:].ap[0], [0, cfg.H]], off_extra=ti),
            op=OP.mult)
        s1loc = slpool.tile([P, cfg.H], BF16, tag="s1loc")
        nc.vector.tensor_tensor(out=s1loc[:], in0=sA[:], in1=sB[:],
                                op=OP.add)

        # broadcast s1[dst] to edge slots: psum_l[:, k, :] = ST_k.T @ s1loc
        psl = lpsum.tile([P, NCH_T, cfg.H], F32, tag="psl")
        for k in range(NCH_T):
            nc.tensor.matmul(out=psl[:, k, :],
                             lhsT=ST[:, k * P:(k + 1) * P],
                             rhs=s1loc[:], start=True, stop=True)

        # logits -> ex
        u = lpool.tile([P, NCH_T, cfg.H], BF16, tag="u")
        nc.vector.tensor_tensor(out=u[:], in0=psl[:],
                                in1=G[:, :, cfg.F:cfg.F + cfg.H], op=OP.add)
        a = lpool.tile([P, NCH_T, cfg.H], BF16, tag="a")
        nc.vector.scalar_tensor_tensor(out=a[:], in0=u[:], scalar=cfg.NEG,
                                       in1=u[:], op0=OP.mult, op1=OP.max)
        ex = lpool.tile([P, NCH_T, cfg.H], BF16, tag="ex")
        nc.scalar.activation(ex[:], a[:], AF.Exp)

        # M = [feat * ex | ex]  (fp8)
        M = mpool.tile([P, NCH_T, MCOLS], FP8, tag="M")
        nc.scalar.activation(M[:, :, cfg.F:MCOLS], ex[:], AF.Copy)
        nc.vector.tensor_tensor(
            out=_ap(M[:], [M[:].ap[0], [MCOLS, NCH_T], [cfg.HID, cfg.H],
                           [1, cfg.HID]]),
            in0=_ap(G[:], [G[:].ap[0], [ROW1, NCH_T], [cfg.HID, cfg.H],
                           [1, cfg.HID]]),
            in1=_ap(ex[:], [ex[:].ap[0], [cfg.H, NCH_T], [1, cfg.H],
                            [0, cfg.HID]]),
            op=OP.mult)

        # aggregate
        agg = apsum.tile([P, MCOLS], F32, tag="agg")
        for k in range(NCH_T):
            nc.tensor.matmul(out=agg[:], lhsT=S[:, k * P:(k + 1) * P],
                             rhs=M[:, k, :],
                             start=(k == 0), stop=(k == NCH_T - 1))

        # normalize + elu + feat2/d2/s2 slab
        den = hpool.tile([P, cfg.H], F32, tag="den")
        nc.vector.tensor_scalar_add(den[:], agg[:, cfg.F:MCOLS], 1e-20)
        rcp = hpool.tile([P, cfg.H], F32, tag="rcp")
        nc.vector.reciprocal(rcp[:], den[:])
        h1 = hpool.tile([P, cfg.F], F32, tag="h1")
        nc.vector.tensor_tensor(
            out=_ap(h1[:], [h1[:].ap[0], [cfg.HID, cfg.H], [1, cfg.HID]]),
            in0=_ap(agg[:], [agg[:].ap[0], [cfg.HID, cfg.H], [1, cfg.HID]]),
            in1=_ap(rcp[:], [rcp[:].ap[0], [1, cfg.H], [0, cfg.HID]]),
            op=OP.mult)
        neg = hpool.tile([P, cfg.F], F32, tag="neg")
        nc.vector.tensor_scalar_min(neg[:], h1[:], 0.0)
        pos = hpool.tile([P, cfg.F], F32, tag="pos")
        nc.vector.tensor_scalar_max(pos[:], h1[:], 0.0)
        een = hpool.tile([P, cfg.F], F32, tag="een")
        nc.scalar.activation(een[:], neg[:], AF.Exp)
        elu = hpool.tile([P, cfg.F], F32, tag="elu")
        nc.vector.scalar_tensor_tensor(out=elu[:], in0=een[:], scalar=-1.0,
                                       in1=pos[:], op0=OP.add, op1=OP.add)
        eT_ps = tpsum.tile([P, P], F32, tag="eT")
        nc.tensor.transpose(out=eT_ps[:], in_=elu[:], identity=ident[:])
        eT = hpool.tile([P, P], BF16, tag="eTs")
        nc.scalar.activation(eT[:], eT_ps[:], AF.Copy)
        t2 = t2psum.tile([P, W2COLS], F32, tag="t2")
        nc.tensor.matmul(out=t2[:], lhsT=eT[:], rhs=w2pack[:],
                         start=True, stop=True)
        t2s = hpool.tile([P, W2COLS], F32, tag="t2s")
        nc.scalar.activation(t2s[:], t2[:], AF.Copy)
        nc.sync.dma_start(out=t["table2slab"][ti * P:(ti + 1) * P, :],
                          in_=t2s[:])


@with_exitstack
def _build_b(ctx, tc, cfg, t):
    nc = tc.nc
    NCE_T, NCO_T, NCB_T, TPC = cfg.NCE_T, cfg.NCO_T, cfg.NCB_T, cfg.TPC
    UC = cfg.OUT + 1                    # 17 used row cols: feat2|d2
    MC = cfg.OUT + 1                    # 17 message cols
    ROW2 = 64                           # table2 row elems (bf16, 128B)
    ROWS2 = 128                         # table_s2 row elems (256B)

    consts = ctx.enter_context(tc.tile_pool(name="consts", bufs=1))
    mlo = consts.tile([P, TPC], BF16)
    nc.sync.dma_start(out=mlo[:], in_=t["m_lo"][:, :])
    mhi = consts.tile([P, TPC], BF16)
    nc.sync.dma_start(out=mhi[:], in_=t["m_hi"][:, :])
    ib_all = consts.tile([P, TPC * NCB_T * 8], I16)
    nc.sync.dma_start(out=ib_all[:], in_=t["srcw_b"][:, :])
    isl_all = consts.tile([P, TPC * 8], I16)
    nc.sync.dma_start(out=isl_all[:], in_=t["s1idx_lo"][:, :])
    ish_all = consts.tile([P, TPC * 8], I16)
    nc.sync.dma_start(out=ish_all[:], in_=t["s1idx_hi"][:, :])

    gpool = ctx.enter_context(tc.tile_pool(name="g2", bufs=2))
    spool = ctx.enter_context(tc.tile_pool(name="soh2", bufs=3))
    slpool = ctx.enter_context(tc.tile_pool(name="sloc2", bufs=2))
    lpool = ctx.enter_context(tc.tile_pool(name="l2", bufs=2))
    mpool = ctx.enter_context(tc.tile_pool(name="m2", bufs=2))
    lpsum = ctx.enter_context(tc.tile_pool(name="lps2", bufs=2,
                                           space="PSUM"))
    apsum = ctx.enter_context(tc.tile_pool(name="aps2", bufs=2,
                                           space="PSUM"))
    opool = ctx.enter_context(tc.tile_pool(name="o", bufs=3))

    tab_pair = _ap(t["table2"][:, :], [[2 * ROW2, cfg.N_PAD // 2],
                                       [1, 2 * ROW2]])
    tabs2_hi = t["table_s2"][cfg.SPLIT:cfg.N_PAD, :]
    q = [0]

    def nextq():
        r = q[0] % NQ
        q[0] += 1
        return r

    for ti in range(TPC):
        G = gpool.tile([P, NCB_T, 2 * ROW2], BF16, tag="G2")
        nc.gpsimd.dma_gather(
            out_ap=G[:], in_ap=tab_pair,
            idxs_ap=ib_all[:, ti * NCB_T * 8:(ti + 1) * NCB_T * 8],
            num_idxs=NCB_T * P, num_idxs_reg=NCB_T * P,
            elem_size=2 * ROW2, single_packet=False, queue_num=nextq())
        s2A = slpool.tile([P, 1, ROWS2], BF16, tag="s2A")
        nc.gpsimd.dma_gather(
            out_ap=s2A[:], in_ap=t["table_s2"][:, :],
            idxs_ap=isl_all[:, ti * 8:(ti + 1) * 8],
            num_idxs=P, num_idxs_reg=P, elem_size=ROWS2,
            single_packet=False, queue_num=nextq())
        s2B = slpool.tile([P, 1, ROWS2], BF16, tag="s2B")
        nc.gpsimd.dma_gather(
            out_ap=s2B[:], in_ap=tabs2_hi,
            idxs_ap=ish_all[:, ti * 8:(ti + 1) * 8],
            num_idxs=P, num_idxs_reg=P, elem_size=ROWS2,
            single_packet=False, queue_num=nextq())

        S = spool.tile([P, NCB_T * P], FP8, tag="SB")
        nc.sync.dma_start(out=S[:], in_=t["S_B"][
            :, ti * NCB_T * P:(ti + 1) * NCB_T * P])
        ST = spool.tile([P, NCB_T * P], FP8, tag="STB")
        nc.sync.dma_start(out=ST[:], in_=t["ST_B"][
            :, ti * NCB_T * P:(ti + 1) * NCB_T * P])

        sA = slpool.tile([P, 1], BF16, tag="sA")
        nc.vector.tensor_tensor(
            out=sA[:], in0=s2A[:, 0, 0:1],
            in1=_ap(mlo[:], [mlo[:].ap[0], [0, 1]], off_extra=ti),
            op=OP.mult)
        sB = slpool.tile([P, 1], BF16, tag="sB")
        nc.vector.tensor_tensor(
            out=sB[:], in0=s2B[:, 0, 0:1],
            in1=_ap(mhi[:], [mhi[:].ap[0], [0, 1]], off_extra=ti),
            op=OP.mult)
        s2loc = slpool.tile([P, 1], BF16, tag="s2loc")
        nc.vector.tensor_tensor(out=s2loc[:], in0=sA[:], in1=sB[:],
                                op=OP.add)

        psl = lpsum.tile([P, NCB_T, 1], F32, tag="psl2")
        for k in range(NCB_T):
            nc.tensor.matmul(out=psl[:, k, :],
                             lhsT=ST[:, k * P:(k + 1) * P],
                             rhs=s2loc[:], start=True, stop=True)

        # logits: u = s2[dst] + d2[src]; parity via static col offset
        u = lpool.tile([P, NCB_T, 1], BF16, tag="u2")
        nc.vector.tensor_tensor(
            out=u[:, 0:NCE_T, :], in0=psl[:, 0:NCE_T, :],
            in1=G[:, 0:NCE_T, cfg.OUT:cfg.OUT + 1], op=OP.add)
        nc.vector.tensor_tensor(
            out=u[:, NCE_T:NCB_T, :], in0=psl[:, NCE_T:NCB_T, :],
            in1=G[:, NCE_T:NCB_T, ROW2 + cfg.OUT:ROW2 + cfg.OUT + 1],
            op=OP.add)
        a = lpool.tile([P, NCB_T, 1], BF16, tag="a2")
        nc.vector.scalar_tensor_tensor(out=a[:], in0=u[:], scalar=cfg.NEG,
                                       in1=u[:], op0=OP.mult, op1=OP.max)
        ex = lpool.tile([P, NCB_T, 1], BF16, tag="ex2")
        nc.scalar.activation(ex[:], a[:], AF.Exp)

        M = mpool.tile([P, NCB_T, MC], FP8, tag="M2")
        nc.scalar.activation(M[:, :, cfg.OUT:MC], ex[:], AF.Copy)
        nc.vector.tensor_tensor(
            out=M[:, 0:NCE_T, 0:cfg.OUT],
            in0=G[:, 0:NCE_T, 0:cfg.OUT],
            in1=_ap(ex[:], [ex[:].ap[0], [1, NCE_T], [0, cfg.OUT]]),
            op=OP.mult)
        nc.vector.tensor_tensor(
            out=M[:, NCE_T:NCB_T, 0:cfg.OUT],
            in0=G[:, NCE_T:NCB_T, ROW2:ROW2 + cfg.OUT],
            in1=_ap(ex[:], [ex[:].ap[0], [1, NCO_T], [0, cfg.OUT]],
                    off_extra=NCE_T),
            op=OP.mult)

        agg = apsum.tile([P, MC], F32, tag="agg2")
        for k in range(NCB_T):
            nc.tensor.matmul(out=agg[:], lhsT=S[:, k * P:(k + 1) * P],
                             rhs=M[:, k, :],
                             start=(k == 0), stop=(k == NCB_T - 1))

        den = opool.tile([P, 1], F32, tag="den")
        nc.vector.tensor_scalar_add(den[:], agg[:, cfg.OUT:MC], 1e-20)
        rcp = opool.tile([P, 1], F32, tag="rcp")
        nc.vector.reciprocal(rcp[:], den[:])
        h2 = opool.tile([P, cfg.OUT], F32, tag="h2")
        nc.vector.tensor_tensor(
            out=h2[:], in0=agg[:, 0:cfg.OUT],
            in1=_ap(rcp[:], [rcp[:].ap[0], [0, cfg.OUT]]), op=OP.mult)
        m = opool.tile([P, 1], F32, tag="m")
        nc.vector.tensor_reduce(out=m[:], in_=h2[:],
                                axis=mybir.AxisListType.X, op=OP.max)
        tm = opool.tile([P, cfg.OUT], F32, tag="tm")
        nc.vector.tensor_tensor(
            out=tm[:], in0=h2[:],
            in1=_ap(m[:], [m[:].ap[0], [0, cfg.OUT]]), op=OP.subtract)
        pe = opool.tile([P, cfg.OUT], F32, tag="pe")
        ssum = opool.tile([P, 1], F32, tag="ss")
        nc.scalar.activation(pe[:], tm[:], AF.Exp, accum_out=ssum[:])
        ln = opool.tile([P, 1], F32, tag="ln")
        nc.scalar.activation(ln[:], ssum[:], AF.Ln)
        res = opool.tile([P, cfg.OUT], F32, tag="res")
        nc.vector.tensor_tensor(
            out=res[:], in0=tm[:],
            in1=_ap(ln[:], [ln[:].ap[0], [0, cfg.OUT]]), op=OP.subtract)
        nc.sync.dma_start(out=t["outp"][ti * P:(ti + 1) * P, :], in_=res[:])


def _decl_a(nc, cfg):
    t = {}
    WCOLS = cfg.F + 2 * cfg.H
    W2COLS = cfg.OUT + 2

    def inp(name, shape, dt):
        t[name] = nc.dram_tensor(name, shape, dt, kind="ExternalInput").ap()

    inp("xT", [P, cfg.N_PAD], BF16)
    inp("wpack", [P, WCOLS], BF16)
    inp("w2pack", [P, W2COLS], BF16)
    inp("ident", [P, P], F32)
    inp("m_lo", [P, cfg.TPC], BF16)
    inp("m_hi", [P, cfg.TPC], BF16)
    inp("srcw_lo", [P, cfg.TPC * cfg.NCHL_T * 8], I16)
    inp("srcw_hi", [P, cfg.TPC * cfg.NCHH_T * 8], I16)
    inp("s1idx_lo", [P, cfg.TPC * 8], I16)
    inp("s1idx_hi", [P, cfg.TPC * 8], I16)
    inp("S_A", [P, cfg.TPC * cfg.NCH_T * P], FP8)
    inp("ST_A", [P, cfg.TPC * cfg.NCH_T * P], FP8)
    t["table1"] = nc.dram_tensor("table1", [cfg.N_PAD, 256], BF16,
                                 kind="Internal").ap()
    t["table_s"] = nc.dram_tensor("table_s", [cfg.N_PAD, 128], BF16,
                                  kind="Internal").ap()
    t["table2slab"] = nc.dram_tensor("table2slab",
                                     [cfg.NPC_PAD, cfg.OUT + 2],
                                     F32, kind="ExternalOutput").ap()
    return t


def _decl_b(nc, cfg):
    t = {}

    def inp(name, shape, dt):
        t[name] = nc.dram_tensor(name, shape, dt, kind="ExternalInput").ap()

    inp("table2", [cfg.N_PAD, 64], BF16)
    inp("table_s2", [cfg.N_PAD, 128], BF16)
    inp("m_lo", [P, cfg.TPC], BF16)
    inp("m_hi", [P, cfg.TPC], BF16)
    inp("srcw_b", [P, cfg.TPC * cfg.NCB_T * 8], I16)
    inp("s1idx_lo", [P, cfg.TPC * 8], I16)
    inp("s1idx_hi", [P, cfg.TPC * 8], I16)
    inp("S_B", [P, cfg.TPC * cfg.NCB_T * P], FP8)
    inp("ST_B", [P, cfg.TPC * cfg.NCB_T * P], FP8)
    t["outp"] = nc.dram_tensor("outp", [cfg.NPC_PAD, cfg.OUT], F32,
                               kind="ExternalOutput").ap()
    return t


def _compile(build_fn, decl_fn, cfg):
    nc = bacc.Bacc("TRN2", target_bir_lowering=False, debug=False,
                   enable_asserts=False, num_devices=cfg.NCORES,
                   num_swdge_queues=NQ)
    t = decl_fn(nc, cfg)
    with tile.TileContext(nc) as tc:
        build_fn(tc, cfg, t)
    nc.compile()
    return nc


def _host_prep_weights(cfg, W1, att_src1, att_dst1, W2, att_src2, att_dst2):
    A_d1 = _blockdiag_att(np.asarray(att_dst1, np.float32), cfg.H, cfg.HID,
                          cfg.F)
    A_s1 = _blockdiag_att(np.asarray(att_src1, np.float32), cfg.H, cfg.HID,
                          cfg.F)
    W1T = np.asarray(W1, np.float32).T.copy()
    wpack = np.concatenate([W1T, W1T @ A_d1, W1T @ A_s1], axis=1)
    W2T = np.asarray(W2, np.float32).T.copy()
    a_d2 = np.asarray(att_dst2, np.float32).reshape(cfg.OUT, 1)
    a_s2 = np.asarray(att_src2, np.float32).reshape(cfg.OUT, 1)
    w2pack = np.concatenate([W2T, W2T @ a_d2, W2T @ a_s2], axis=1)
    return (np.ascontiguousarray(wpack.astype(BF)),
            np.ascontiguousarray(w2pack.astype(BF)))


_CACHE = {}


def _get_kernels(cfg):
    key = (cfg.N, cfg.E, cfg.NCORES, cfg.NCHL_T, cfg.NCHH_T,
           cfg.NCE_T, cfg.NCO_T)
    if key not in _CACHE:
        nca = _compile(_build_a, _decl_a, cfg)
        ncb = _compile(_build_b, _decl_b, cfg)
        _CACHE[key] = (nca, ncb)
    return _CACHE[key]


def run(cfg, inputs, runner=None):
    x = np.asarray(inputs["x"], np.float32)
    edge_index = np.asarray(inputs["edge_index"], np.int64)
    pc = _prep_graph(cfg, edge_index)
    wpack, w2pack = _host_prep_weights(
        cfg, inputs["W1"], inputs["att_src1"], inputs["att_dst1"],
        inputs["W2"], inputs["att_src2"], inputs["att_dst2"])

    xT = np.zeros((P, cfg.N_PAD), BF)
    xT[:, :cfg.N] = x.T.astype(BF)
    ident = np.eye(P, dtype=np.float32)

    nca, ncb = _get_kernels(cfg)

    if runner is None:
        def runner(nc, in_maps):
            r = bass_utils.run_bass_kernel_spmd(
                nc, in_maps, core_ids=list(range(cfg.NCORES)))
            return r.results

    in_maps_a = []
    for c in range(cfg.NCORES):
        in_maps_a.append(dict(
            xT=xT, wpack=wpack, w2pack=w2pack, ident=ident,
            m_lo=pc["m_lo"][c], m_hi=pc["m_hi"][c],
            srcw_lo=pc["srcw_lo"][c], srcw_hi=pc["srcw_hi"][c],
            s1idx_lo=pc["s1idx_lo"][c], s1idx_hi=pc["s1idx_hi"][c],
            S_A=pc["S_A"][c], ST_A=pc["ST_A"][c]))
    res_a = runner(nca, in_maps_a)

    # host gather: assemble layer-2 tables
    W2C = cfg.OUT + 2
    slab_all = np.zeros((cfg.N_PAD, W2C), np.float32)
    for c in range(cfg.NCORES):
        slab = np.asarray(res_a[c]["table2slab"], np.float32)
        slab_all[c * cfg.NPC:(c + 1) * cfg.NPC] = slab[:cfg.NPC]
    table2 = np.zeros((cfg.N_PAD, 64), BF)
    table2[:, 0:cfg.OUT] = slab_all[:, 0:cfg.OUT].astype(BF)
    table2[:, cfg.OUT] = slab_all[:, cfg.OUT].astype(BF)       # d2
    table_s2 = np.zeros((cfg.N_PAD, 128), BF)
    table_s2[:, 0] = slab_all[:, cfg.OUT + 1].astype(BF)       # s2

    in_maps_b = []
    for c in range(cfg.NCORES):
        in_maps_b.append(dict(
            table2=table2, table_s2=table_s2,
            m_lo=pc["m_lo"][c], m_hi=pc["m_hi"][c],
            srcw_b=pc["srcw_b"][c],
            s1idx_lo=pc["s1idx_lo"][c], s1idx_hi=pc["s1idx_hi"][c],
            S_B=pc["S_B"][c], ST_B=pc["ST_B"][c]))
    res_b = runner(ncb, in_maps_b)

    out = np.zeros((cfg.N, cfg.OUT), np.float32)
    for c in range(cfg.NCORES):
        out[c * cfg.NPC:(c + 1) * cfg.NPC] = \
            np.asarray(res_b[c]["outp"], np.float32)[:cfg.NPC]
    return out


def kernel(**inputs):
    cfg = Cfg(N=50000, E=1600000, ncores=8)
    return run(cfg, inputs)


# revision 8
# speedup vs baseline: 2.6715x; 1.0755x over previous
"""
2-layer GAT on Trainium2 (8 NeuronCores, SPMD via bass/Tile) — v2.

Sharding: destination nodes block-sharded across 8 cores (6250 each).
All per-edge work runs on the core owning the edge's dst.  Layer-0
node-level compute (h = x @ W1pack, bf16) is replicated on every core.
Two kernels (A: layer 1, B: layer 2) with a host gather of per-core
node tables in between.

Key performance structure (vs v1 baseline, 8.29 ms -> target ~1.3 ms):
  - 4 SWDGE queues (num_swdge_queues=4): dma_gather descriptor
    generation parallelises over all 4 Q7 cpu pairs (2.33 ns/idx vs
    7.92 measured).
  - ONE gather index per edge per layer: the dst-side attention scalar
    (s1/s2) is no longer gathered per edge.  Instead each dst tile
    fetches its 128 nodes' scalar with two tiny 128-idx gathers
    (lo/hi of the node id, masked combine), then broadcasts to edge
    slots with a per-chunk PE matmul  lhsT=ST (transposed one-hot).
  - One-hot matrices S (slot->dstlocal, aggregation lhsT) and ST
    (broadcast lhsT) are precomputed on host as fp8 and streamed.
  - Messages M are fp8 (keeps the big DVE mult in 1x mode: no
    GPSIMD/SWDGE port-pair lock), aggregation matmul fp8 x fp8.
  - Node phase in bf16 (fp32 matmul is 4x slower on PE).
"""

import os
import sys

import numpy as np
import ml_dtypes

for _p in ("/opt/trn_rl_repo",):
    if os.path.isdir(_p) and _p not in sys.path:
        sys.path.insert(0, _p)

import concourse.bass as bass
import concourse.bacc as bacc
import concourse.tile as tile
from concourse import mybir
from concourse import bass_utils
from concourse._compat import with_exitstack
from contextlib import ExitStack

F32 = mybir.dt.float32
BF16 = mybir.dt.bfloat16
FP8 = mybir.dt.float8e4
I32 = mybir.dt.int32
I16 = mybir.dt.int16
AF = mybir.ActivationFunctionType
OP = mybir.AluOpType
P = 128
BF = ml_dtypes.bfloat16
F8 = ml_dtypes.float8_e4m3
NQ = 4                     # SWDGE queues


class Cfg:
    def __init__(self, N, E, ncores, split=32768, neg=0.2, in_ch=128,
                 f=128, heads=8, hid=16, out=16):
        self.N = N
        self.E = E
        self.NCORES = ncores
        self.SPLIT = split
        self.NEG = neg
        self.IN = in_ch
        self.F = f
        self.H = heads
        self.HID = hid
        self.OUT = out
        assert N % ncores == 0
        self.NPC = N // ncores
        self.TPC = (self.NPC + P - 1) // P
        self.NPC_PAD = self.TPC * P
        self.NTILES = ncores * self.TPC
        self.N_PAD = self.NTILES * P
        # filled by _prep_graph
        self.NCHL_T = None   # lo chunks per tile (layer A)
        self.NCHH_T = None   # hi chunks per tile (layer A)
        self.NCH_T = None
        self.NCE_T = None    # even-src chunks per tile (layer B)
        self.NCO_T = None    # odd-src chunks per tile (layer B)
        self.NCB_T = None


def _wrap16(vals):
    """[n] slot-ordered int idx -> [128, n//16] int16 wrapped layout."""
    n = vals.shape[0]
    assert n % 16 == 0
    w = vals.reshape(-1, 16).T.astype(np.int16)
    return np.ascontiguousarray(np.tile(w, (8, 1)))


def _slot_fill(src_vals, dloc_vals, pos, nch_grp, grp_off, t, NCH_T, v_idx,
               s_rows, s_cols, st_rows, st_cols):
    """Record slot assignments for one (tile, group) run of edges."""
    chunk = t * NCH_T + grp_off + pos // P
    part = pos % P
    v_idx[...] = src_vals
    s_rows.append(part)
    s_cols.append(chunk * P + dloc_vals)
    st_rows.append(dloc_vals)
    st_cols.append(chunk * P + part)


def _prep_graph(cfg, edge_index):
    N, NPC, TPC, SPL = cfg.N, cfg.NPC, cfg.TPC, cfg.SPLIT
    src = np.concatenate([edge_index[0], np.arange(N, dtype=np.int64)])
    dst = np.concatenate([edge_index[1], np.arange(N, dtype=np.int64)])
    core = dst // NPC
    ld = dst - core * NPC
    tile_id = ld // P
    dloc = ld % P

    # ---- layer A grouping: (core, tile, hi(src), src) ----
    hi = (src >= SPL).astype(np.int64)
    orderA = np.lexsort((src, hi, tile_id, core))
    keyA = (core * TPC + tile_id) * 2 + hi
    cntA = np.bincount(keyA, minlength=cfg.NCORES * TPC * 2)
    cnt_lo = cntA[0::2].reshape(cfg.NCORES, TPC)
    cnt_hi = cntA[1::2].reshape(cfg.NCORES, TPC)
    cfg.NCHL_T = max(1, int(np.max((cnt_lo + P - 1) // P)))
    cfg.NCHH_T = max(1, int(np.max((cnt_hi + P - 1) // P)))
    cfg.NCH_T = cfg.NCHL_T + cfg.NCHH_T

    # ---- layer B grouping: (core, tile, parity(src), src) ----
    par = (src & 1).astype(np.int64)
    orderB = np.lexsort((src, par, tile_id, core))
    keyB = (core * TPC + tile_id) * 2 + par
    cntB = np.bincount(keyB, minlength=cfg.NCORES * TPC * 2)
    cnt_ev = cntB[0::2].reshape(cfg.NCORES, TPC)
    cnt_od = cntB[1::2].reshape(cfg.NCORES, TPC)
    cfg.NCE_T = max(1, int(np.max((cnt_ev + P - 1) // P)))
    cfg.NCO_T = max(1, int(np.max((cnt_od + P - 1) // P)))
    cfg.NCB_T = cfg.NCE_T + cfg.NCO_T

    startsA = np.concatenate([[0], np.cumsum(cntA)])
    startsB = np.concatenate([[0], np.cumsum(cntB)])
    ONE = np.uint8(0x38)  # 1.0 in float8_e4m3

    pc = dict(srcw_lo=[], srcw_hi=[], srcw_b=[], S_A=[], ST_A=[],
              S_B=[], ST_B=[], s1idx_lo=[], s1idx_hi=[], m_lo=[], m_hi=[])
    sA, dA = src[orderA], dloc[orderA]
    sB, dB = src[orderB], dloc[orderB]
    for c in range(cfg.NCORES):
        v_lo = np.zeros(TPC * cfg.NCHL_T * P, np.int64)
        v_hi = np.zeros(TPC * cfg.NCHH_T * P, np.int64)
        v_b = np.zeros(TPC * cfg.NCB_T * P, np.int64)
        SA = np.zeros((P, TPC * cfg.NCH_T * P), np.uint8)
        STA = np.zeros((P, TPC * cfg.NCH_T * P), np.uint8)
        SB = np.zeros((P, TPC * cfg.NCB_T * P), np.uint8)
        STB = np.zeros((P, TPC * cfg.NCB_T * P), np.uint8)
        for t in range(TPC):
            for g in (0, 1):
                # layer A
                k = (c * TPC + t) * 2 + g
                n = int(cntA[k])
                if n:
                    sl = slice(startsA[k], startsA[k] + n)
                    e_src, e_dl = sA[sl], dA[sl]
                    pos = np.arange(n)
                    if g == 0:
                        v_lo[t * cfg.NCHL_T * P + pos] = e_src
                        chunk = t * cfg.NCH_T + pos // P
                    else:
                        v_hi[t * cfg.NCHH_T * P + pos] = e_src - SPL
                        chunk = t * cfg.NCH_T + cfg.NCHL_T + pos // P
                    part = pos % P
                    SA[part, chunk * P + e_dl] = ONE
                    STA[e_dl, chunk * P + part] = ONE
                # layer B
                n = int(cntB[k])
                if n:
                    sl = slice(startsB[k], startsB[k] + n)
                    e_src, e_dl = sB[sl], dB[sl]
                    pos = np.arange(n)
                    if g == 0:
                        chunk = t * cfg.NCB_T + pos // P
                    else:
                        chunk = t * cfg.NCB_T + cfg.NCE_T + pos // P
                    v_b[chunk * P + pos % P] = e_src >> 1
                    part = pos % P
                    SB[part, chunk * P + e_dl] = ONE
                    STB[e_dl, chunk * P + part] = ONE
        pc["srcw_lo"].append(_wrap16(v_lo))
        pc["srcw_hi"].append(_wrap16(v_hi))
        pc["srcw_b"].append(_wrap16(v_b))
        pc["S_A"].append(SA.view(F8))
        pc["ST_A"].append(STA.view(F8))
        pc["S_B"].append(SB.view(F8))
        pc["ST_B"].append(STB.view(F8))
        # dst-tile node-id gathers for s1/s2 (lo/hi + mask combine)
        nodes = c * NPC + np.arange(cfg.NPC_PAD, dtype=np.int64)
        lo_sel = nodes < SPL
        idx_lo = np.where(lo_sel, nodes, 0)
        idx_hi = np.where(lo_sel, 0, nodes - SPL)
        pc["s1idx_lo"].append(_wrap16(idx_lo))
        pc["s1idx_hi"].append(_wrap16(idx_hi))
        m = lo_sel.astype(np.float32).reshape(TPC, P).T   # [128, TPC]
        pc["m_lo"].append(np.ascontiguousarray(m.astype(BF)))
        pc["m_hi"].append(np.ascontiguousarray((1.0 - m).astype(BF)))
    return pc


def _blockdiag_att(att, heads, hid, f):
    A = np.zeros((f, heads), dtype=np.float32)
    for h in range(heads):
        A[h * hid:(h + 1) * hid, h] = att[0, h]
    return A


def _ap(base, ap_list, off_extra=0):
    return bass.AP(tensor=base.tensor, offset=base.offset + off_extra,
                   ap=ap_list)


@with_exitstack
def _build_a(ctx, tc, cfg, t):
    nc = tc.nc
    NCHL_T, NCHH_T, NCH_T, TPC = cfg.NCHL_T, cfg.NCHH_T, cfg.NCH_T, cfg.TPC
    WCOLS = cfg.F + 2 * cfg.H             # 144 matmul out cols
    TCOLS = cfg.F + cfg.H                 # 136 table1 used cols
    MCOLS = cfg.F + cfg.H                 # 136 message cols
    ROW1 = 256                            # table1 row elems (bf16, 512B)
    ROWS = 128                            # table_s row elems (bf16, 256B)
    W2COLS = cfg.OUT + 2

    consts = ctx.enter_context(tc.tile_pool(name="consts", bufs=1))
    wpack = consts.tile([P, WCOLS], BF16)
    nc.sync.dma_start(out=wpack[:], in_=t["wpack"][:, :])
    w2pack = consts.tile([P, W2COLS], BF16)
    nc.sync.dma_start(out=w2pack[:], in_=t["w2pack"][:, :])
    ident = consts.tile([P, P], F32)
    nc.sync.dma_start(out=ident[:], in_=t["ident"][:, :])
    mlo = consts.tile([P, TPC], BF16)
    nc.sync.dma_start(out=mlo[:], in_=t["m_lo"][:, :])
    mhi = consts.tile([P, TPC], BF16)
    nc.sync.dma_start(out=mhi[:], in_=t["m_hi"][:, :])
    # all idx tiles loaded once
    il_all = consts.tile([P, TPC * NCHL_T * 8], I16)
    nc.sync.dma_start(out=il_all[:], in_=t["srcw_lo"][:, :])
    ih_all = consts.tile([P, TPC * NCHH_T * 8], I16)
    nc.sync.dma_start(out=ih_all[:], in_=t["srcw_hi"][:, :])
    isl_all = consts.tile([P, TPC * 8], I16)
    nc.sync.dma_start(out=isl_all[:], in_=t["s1idx_lo"][:, :])
    ish_all = consts.tile([P, TPC * 8], I16)
    nc.sync.dma_start(out=ish_all[:], in_=t["s1idx_hi"][:, :])

    # ---------------- node phase (bf16) ----------------
    NT = cfg.NTILES
    BLK = 8
    with ExitStack() as nctx:
        xpool = nctx.enter_context(tc.tile_pool(name="xt", bufs=2))
        npsum = nctx.enter_context(tc.tile_pool(name="npsum", bufs=4,
                                                space="PSUM"))
        nstage = nctx.enter_context(tc.tile_pool(name="nstage", bufs=3))
        for blk in range((NT + BLK - 1) // BLK):
            nt0 = blk * BLK
            nt1 = min(nt0 + BLK, NT)
            xt = xpool.tile([P, BLK * P], BF16, tag="xt")
            nc.sync.dma_start(out=xt[:, 0:(nt1 - nt0) * P],
                              in_=t["xT"][:, nt0 * P:nt1 * P])
            for j in range(nt1 - nt0):
                nt = nt0 + j
                pt = npsum.tile([P, WCOLS], F32, tag="npt")
                nc.tensor.matmul(out=pt[:], lhsT=xt[:, j * P:(j + 1) * P],
                                 rhs=wpack[:], start=True, stop=True)
                s1 = nstage.tile([P, TCOLS], BF16, tag="s1")
                nc.scalar.activation(s1[:], pt[:, 0:TCOLS], AF.Copy)
                ss = nstage.tile([P, cfg.H], BF16, tag="ss")
                nc.scalar.activation(ss[:], pt[:, TCOLS:WCOLS], AF.Copy)
                nc.sync.dma_start(
                    out=t["table1"][nt * P:(nt + 1) * P, 0:TCOLS], in_=s1[:])
                nc.sync.dma_start(
                    out=t["table_s"][nt * P:(nt + 1) * P, 0:cfg.H],
                    in_=ss[:])

    # Tile does not track DRAM deps: fence table writes vs gathers.
    tc.strict_bb_all_engine_barrier()

    # ---------------- edge phase ----------------
    gpool = ctx.enter_context(tc.tile_pool(name="g", bufs=3))
    spool = ctx.enter_context(tc.tile_pool(name="soh", bufs=6))
    slpool = ctx.enter_context(tc.tile_pool(name="sloc", bufs=4))
    lpool = ctx.enter_context(tc.tile_pool(name="logit", bufs=3))
    mpool = ctx.enter_context(tc.tile_pool(name="msg", bufs=3))
    lpsum = ctx.enter_context(tc.tile_pool(name="lpsum", bufs=2,
                                           space="PSUM"))
    apsum = ctx.enter_context(tc.tile_pool(name="apsum", bufs=2,
                                           space="PSUM"))
    tpsum = ctx.enter_context(tc.tile_pool(name="tpsum", bufs=2,
                                           space="PSUM"))
    t2psum = ctx.enter_context(tc.tile_pool(name="t2psum", bufs=2,
                                            space="PSUM"))
    hpool = ctx.enter_context(tc.tile_pool(name="h1", bufs=2))

    tab_hi = t["table1"][cfg.SPLIT:cfg.N_PAD, :]
    tabs_hi = t["table_s"][cfg.SPLIT:cfg.N_PAD, :]

    for ti in range(TPC):
        q = [ti]

        def nextq():
            r = q[0] % NQ
            q[0] += 1
            return r
        # gathers: feat+d1 rows by src (lo/hi), s1 rows for own nodes
        G = gpool.tile([P, NCH_T, ROW1], BF16, tag="G")
        nc.gpsimd.dma_gather(
            out_ap=G[:, 0:NCHL_T, :], in_ap=t["table1"][:, :],
            idxs_ap=il_all[:, ti * NCHL_T * 8:(ti + 1) * NCHL_T * 8],
            num_idxs=NCHL_T * P, num_idxs_reg=NCHL_T * P,
            elem_size=ROW1, single_packet=False, queue_num=nextq())
        nc.gpsimd.dma_gather(
            out_ap=G[:, NCHL_T:NCH_T, :], in_ap=tab_hi,
            idxs_ap=ih_all[:, ti * NCHH_T * 8:(ti + 1) * NCHH_T * 8],
            num_idxs=NCHH_T * P, num_idxs_reg=NCHH_T * P,
            elem_size=ROW1, single_packet=False, queue_num=nextq())
        s1A = slpool.tile([P, 1, ROWS], BF16, tag="s1A")
        nc.gpsimd.dma_gather(
            out_ap=s1A[:], in_ap=t["table_s"][:, :],
            idxs_ap=isl_all[:, ti * 8:(ti + 1) * 8],
            num_idxs=P, num_idxs_reg=P, elem_size=ROWS,
            single_packet=False, queue_num=nextq())
        s1B = slpool.tile([P, 1, ROWS], BF16, tag="s1B")
        nc.gpsimd.dma_gather(
            out_ap=s1B[:], in_ap=tabs_hi,
            idxs_ap=ish_all[:, ti * 8:(ti + 1) * 8],
            num_idxs=P, num_idxs_reg=P, elem_size=ROWS,
            single_packet=False, queue_num=nextq())

        # one-hot streams
        S = spool.tile([P, NCH_T * P], FP8, tag="S")
        nc.sync.dma_start(out=S[:], in_=t["S_A"][
            :, ti * NCH_T * P:(ti + 1) * NCH_T * P])
        ST = spool.tile([P, NCH_T * P], FP8, tag="ST")
        nc.sync.dma_start(out=ST[:], in_=t["ST_A"][
            :, ti * NCH_T * P:(ti + 1) * NCH_T * P])

        # s1loc[j, :] = s1 of the tile's j-th node (lo/hi masked)
        sA = slpool.tile([P, cfg.H], BF16, tag="sA")
        nc.vector.tensor_tensor(
            out=sA[:], in0=s1A[:, 0, 0:cfg.H],
            in1=_ap(mlo[:], [mlo[:].ap[0], [0, cfg.H]], off_extra=ti),
            op=OP.mult)
        sB = slpool.tile([P, cfg.H], BF16, tag="sB")
        nc.vector.tensor_tensor(
            out=sB[:], in0=s1B[:, 0, 0:cfg.H],
            in1=_ap(mhi[:], [mhi[:].ap[0], [0, cfg.H]], off_extra=ti),
            op=OP.mult)
        s1loc = slpool.tile([P, cfg.H], BF16, tag="s1loc")
        nc.vector.tensor_tensor(out=s1loc[:], in0=sA[:], in1=sB[:],
                                op=OP.add)

        # broadcast s1[dst] to edge slots: psum_l[:, k, :] = ST_k.T @ s1loc
        psl = lpsum.tile([P, NCH_T, cfg.H], F32, tag="psl")
        for k in range(NCH_T):
            nc.tensor.matmul(out=psl[:, k, :],
                             lhsT=ST[:, k * P:(k + 1) * P],
                             rhs=s1loc[:], start=True, stop=True)

        # logits -> ex
        u = lpool.tile([P, NCH_T, cfg.H], BF16, tag="u")
        nc.vector.tensor_tensor(out=u[:], in0=psl[:],
                                in1=G[:, :, cfg.F:cfg.F + cfg.H], op=OP.add)
        a = lpool.tile([P, NCH_T, cfg.H], BF16, tag="a")
        nc.vector.scalar_tensor_tensor(out=a[:], in0=u[:], scalar=cfg.NEG,
                                       in1=u[:], op0=OP.mult, op1=OP.max)
        ex = lpool.tile([P, NCH_T, cfg.H], BF16, tag="ex")
        nc.scalar.activation(ex[:], a[:], AF.Exp)

        # M = [feat * ex | ex]  (fp8)
        M = mpool.tile([P, NCH_T, MCOLS], FP8, tag="M")
        nc.scalar.activation(M[:, :, cfg.F:MCOLS], ex[:], AF.Copy)
        nc.vector.tensor_tensor(
            out=_ap(M[:], [M[:].ap[0], [MCOLS, NCH_T], [cfg.HID, cfg.H],
                           [1, cfg.HID]]),
            in0=_ap(G[:], [G[:].ap[0], [ROW1, NCH_T], [cfg.HID, cfg.H],
                           [1, cfg.HID]]),
            in1=_ap(ex[:], [ex[:].ap[0], [cfg.H, NCH_T], [1, cfg.H],
                            [0, cfg.HID]]),
            op=OP.mult)

        # aggregate
        agg = apsum.tile([P, MCOLS], F32, tag="agg")
        for k in range(NCH_T):
            nc.tensor.matmul(out=agg[:], lhsT=S[:, k * P:(k + 1) * P],
                             rhs=M[:, k, :],
                             start=(k == 0), stop=(k == NCH_T - 1))

        # normalize + elu + feat2/d2/s2 slab
        den = hpool.tile([P, cfg.H], F32, tag="den")
        nc.vector.tensor_scalar_add(den[:], agg[:, cfg.F:MCOLS], 1e-20)
        rcp = hpool.tile([P, cfg.H], F32, tag="rcp")
        nc.vector.reciprocal(rcp[:], den[:])
        h1 = hpool.tile([P, cfg.F], F32, tag="h1")
        nc.vector.tensor_tensor(
            out=_ap(h1[:], [h1[:].ap[0], [cfg.HID, cfg.H], [1, cfg.HID]]),
            in0=_ap(agg[:], [agg[:].ap[0], [cfg.HID, cfg.H], [1, cfg.HID]]),
            in1=_ap(rcp[:], [rcp[:].ap[0], [1, cfg.H], [0, cfg.HID]]),
            op=OP.mult)
        pos = hpool.tile([P, cfg.F], F32, tag="pos")
        nc.scalar.activation(pos[:], h1[:], AF.Relu)
        nr = hpool.tile([P, cfg.F], F32, tag="nr")
        nc.scalar.activation(nr[:], h1[:], AF.Relu, scale=-1.0)
        een = hpool.tile([P, cfg.F], F32, tag="een")
        nc.scalar.activation(een[:], nr[:], AF.Exp, scale=-1.0)
        elu = hpool.tile([P, cfg.F], F32, tag="elu")
        nc.vector.scalar_tensor_tensor(out=elu[:], in0=een[:], scalar=-1.0,
                                       in1=pos[:], op0=OP.add, op1=OP.add)
        eT_ps = tpsum.tile([P, P], F32, tag="eT")
        nc.tensor.transpose(out=eT_ps[:], in_=elu[:], identity=ident[:])
        eT = hpool.tile([P, P], BF16, tag="eTs")
        nc.scalar.activation(eT[:], eT_ps[:], AF.Copy)
        t2 = t2psum.tile([P, W2COLS], F32, tag="t2")
        nc.tensor.matmul(out=t2[:], lhsT=eT[:], rhs=w2pack[:],
                         start=True, stop=True)
        t2s = hpool.tile([P, W2COLS], F32, tag="t2s")
        nc.scalar.activation(t2s[:], t2[:], AF.Copy)
        nc.sync.dma_start(out=t["table2slab"][ti * P:(ti + 1) * P, :],
                          in_=t2s[:])


@with_exitstack
def _build_b(ctx, tc, cfg, t):
    nc = tc.nc
    NCE_T, NCO_T, NCB_T, TPC = cfg.NCE_T, cfg.NCO_T, cfg.NCB_T, cfg.TPC
    UC = cfg.OUT + 1                    # 17 used row cols: feat2|d2
    MC = cfg.OUT + 1                    # 17 message cols
    ROW2 = 64                           # table2 row elems (bf16, 128B)
    ROWS2 = 128                         # table_s2 row elems (256B)

    consts = ctx.enter_context(tc.tile_pool(name="consts", bufs=1))
    mlo = consts.tile([P, TPC], BF16)
    nc.sync.dma_start(out=mlo[:], in_=t["m_lo"][:, :])
    mhi = consts.tile([P, TPC], BF16)
    nc.sync.dma_start(out=mhi[:], in_=t["m_hi"][:, :])
    ib_all = consts.tile([P, TPC * NCB_T * 8], I16)
    nc.sync.dma_start(out=ib_all[:], in_=t["srcw_b"][:, :])
    isl_all = consts.tile([P, TPC * 8], I16)
    nc.sync.dma_start(out=isl_all[:], in_=t["s1idx_lo"][:, :])
    ish_all = consts.tile([P, TPC * 8], I16)
    nc.sync.dma_start(out=ish_all[:], in_=t["s1idx_hi"][:, :])

    gpool = ctx.enter_context(tc.tile_pool(name="g2", bufs=4))
    spool = ctx.enter_context(tc.tile_pool(name="soh2", bufs=8))
    slpool = ctx.enter_context(tc.tile_pool(name="sloc2", bufs=4))
    lpool = ctx.enter_context(tc.tile_pool(name="l2", bufs=3))
    mpool = ctx.enter_context(tc.tile_pool(name="m2", bufs=3))
    lpsum = ctx.enter_context(tc.tile_pool(name="lps2", bufs=2,
                                           space="PSUM"))
    apsum = ctx.enter_context(tc.tile_pool(name="aps2", bufs=2,
                                           space="PSUM"))
    opool = ctx.enter_context(tc.tile_pool(name="o", bufs=3))
    tailp = ctx.enter_context(tc.tile_pool(name="tail", bufs=1))
    h2all = tailp.tile([P, TPC, cfg.OUT], F32)

    tab_pair = _ap(t["table2"][:, :], [[2 * ROW2, cfg.N_PAD // 2],
                                       [1, 2 * ROW2]])
    tabs2_hi = t["table_s2"][cfg.SPLIT:cfg.N_PAD, :]

    for ti in range(TPC):
        q = [ti]

        def nextq():
            r = q[0] % NQ
            q[0] += 1
            return r
        G = gpool.tile([P, NCB_T, 2 * ROW2], BF16, tag="G2")
        nc.gpsimd.dma_gather(
            out_ap=G[:], in_ap=tab_pair,
            idxs_ap=ib_all[:, ti * NCB_T * 8:(ti + 1) * NCB_T * 8],
            num_idxs=NCB_T * P, num_idxs_reg=NCB_T * P,
            elem_size=2 * ROW2, single_packet=False, queue_num=nextq())
        s2A = slpool.tile([P, 1, ROWS2], BF16, tag="s2A")
        nc.gpsimd.dma_gather(
            out_ap=s2A[:], in_ap=t["table_s2"][:, :],
            idxs_ap=isl_all[:, ti * 8:(ti + 1) * 8],
            num_idxs=P, num_idxs_reg=P, elem_size=ROWS2,
            single_packet=False, queue_num=nextq())
        s2B = slpool.tile([P, 1, ROWS2], BF16, tag="s2B")
        nc.gpsimd.dma_gather(
            out_ap=s2B[:], in_ap=tabs2_hi,
            idxs_ap=ish_all[:, ti * 8:(ti + 1) * 8],
            num_idxs=P, num_idxs_reg=P, elem_size=ROWS2,
            single_packet=False, queue_num=nextq())

        S = spool.tile([P, NCB_T * P], FP8, tag="SB")
        nc.sync.dma_start(out=S[:], in_=t["S_B"][
            :, ti * NCB_T * P:(ti + 1) * NCB_T * P])
        ST = spool.tile([P, NCB_T * P], FP8, tag="STB")
        nc.sync.dma_start(out=ST[:], in_=t["ST_B"][
            :, ti * NCB_T * P:(ti + 1) * NCB_T * P])

        sA = slpool.tile([P, 1], BF16, tag="sA")
        nc.vector.tensor_tensor(
            out=sA[:], in0=s2A[:, 0, 0:1],
            in1=_ap(mlo[:], [mlo[:].ap[0], [0, 1]], off_extra=ti),
            op=OP.mult)
        sB = slpool.tile([P, 1], BF16, tag="sB")
        nc.vector.tensor_tensor(
            out=sB[:], in0=s2B[:, 0, 0:1],
            in1=_ap(mhi[:], [mhi[:].ap[0], [0, 1]], off_extra=ti),
            op=OP.mult)
        s2loc = slpool.tile([P, 1], BF16, tag="s2loc")
        nc.vector.tensor_tensor(out=s2loc[:], in0=sA[:], in1=sB[:],
                                op=OP.add)

        psl = lpsum.tile([P, NCB_T, 1], F32, tag="psl2")
        for k in range(NCB_T):
            nc.tensor.matmul(out=psl[:, k, :],
                             lhsT=ST[:, k * P:(k + 1) * P],
                             rhs=s2loc[:], start=True, stop=True)

        # logits: u = s2[dst] + d2[src]; parity via static col offset
        u = lpool.tile([P, NCB_T, 1], BF16, tag="u2")
        nc.vector.tensor_tensor(
            out=u[:, 0:NCE_T, :], in0=psl[:, 0:NCE_T, :],
            in1=G[:, 0:NCE_T, cfg.OUT:cfg.OUT + 1], op=OP.add)
        nc.vector.tensor_tensor(
            out=u[:, NCE_T:NCB_T, :], in0=psl[:, NCE_T:NCB_T, :],
            in1=G[:, NCE_T:NCB_T, ROW2 + cfg.OUT:ROW2 + cfg.OUT + 1],
            op=OP.add)
        a = lpool.tile([P, NCB_T, 1], BF16, tag="a2")
        nc.vector.scalar_tensor_tensor(out=a[:], in0=u[:], scalar=cfg.NEG,
                                       in1=u[:], op0=OP.mult, op1=OP.max)
        ex = lpool.tile([P, NCB_T, 1], BF16, tag="ex2")
        nc.scalar.activation(ex[:], a[:], AF.Exp)

        M = mpool.tile([P, NCB_T, MC], FP8, tag="M2")
        nc.scalar.activation(M[:, :, cfg.OUT:MC], ex[:], AF.Copy)
        nc.vector.tensor_tensor(
            out=M[:, 0:NCE_T, 0:cfg.OUT],
            in0=G[:, 0:NCE_T, 0:cfg.OUT],
            in1=_ap(ex[:], [ex[:].ap[0], [1, NCE_T], [0, cfg.OUT]]),
            op=OP.mult)
        nc.vector.tensor_tensor(
            out=M[:, NCE_T:NCB_T, 0:cfg.OUT],
            in0=G[:, NCE_T:NCB_T, ROW2:ROW2 + cfg.OUT],
            in1=_ap(ex[:], [ex[:].ap[0], [1, NCO_T], [0, cfg.OUT]],
                    off_extra=NCE_T),
            op=OP.mult)

        agg = apsum.tile([P, MC], F32, tag="agg2")
        for k in range(NCB_T):
            nc.tensor.matmul(out=agg[:], lhsT=S[:, k * P:(k + 1) * P],
                             rhs=M[:, k, :],
                             start=(k == 0), stop=(k == NCB_T - 1))

        den = opool.tile([P, 1], F32, tag="den")
        nc.vector.tensor_scalar_add(den[:], agg[:, cfg.OUT:MC], 1e-20)
        rcp = opool.tile([P, 1], F32, tag="rcp")
        nc.vector.reciprocal(rcp[:], den[:])
        nc.vector.tensor_tensor(
            out=h2all[:, ti, :], in0=agg[:, 0:cfg.OUT],
            in1=_ap(rcp[:], [rcp[:].ap[0], [0, cfg.OUT]]), op=OP.mult)

    # batched log_softmax tail: one Exp + one Ln pass (avoids per-tile
    # activation-table reloads between Exp and Ln)
    m = tailp.tile([P, TPC], F32)
    nc.vector.tensor_reduce(out=m[:], in_=h2all[:],
                            axis=mybir.AxisListType.X, op=OP.max)
    tm = tailp.tile([P, TPC, cfg.OUT], F32)
    nc.vector.tensor_tensor(
        out=tm[:], in0=h2all[:],
        in1=_ap(m[:], [m[:].ap[0], [1, TPC], [0, cfg.OUT]]),
        op=OP.subtract)
    pe = tailp.tile([P, TPC, cfg.OUT], F32)
    nc.scalar.activation(pe[:], tm[:], AF.Exp)
    ssum = tailp.tile([P, TPC], F32)
    nc.vector.tensor_reduce(out=ssum[:], in_=pe[:],
                            axis=mybir.AxisListType.X, op=OP.add)
    ln = tailp.tile([P, TPC], F32)
    nc.scalar.activation(ln[:], ssum[:], AF.Ln)
    res = tailp.tile([P, TPC, cfg.OUT], F32)
    nc.vector.tensor_tensor(
        out=res[:], in0=tm[:],
        in1=_ap(ln[:], [ln[:].ap[0], [1, TPC], [0, cfg.OUT]]),
        op=OP.subtract)
    for ti in range(TPC):
        nc.sync.dma_start(out=t["outp"][ti * P:(ti + 1) * P, :],
                          in_=res[:, ti, :])


def _decl_a(nc, cfg):
    t = {}
    WCOLS = cfg.F + 2 * cfg.H
    W2COLS = cfg.OUT + 2

    def inp(name, shape, dt):
        t[name] = nc.dram_tensor(name, shape, dt, kind="ExternalInput").ap()

    inp("xT", [P, cfg.N_PAD], BF16)
    inp("wpack", [P, WCOLS], BF16)
    inp("w2pack", [P, W2COLS], BF16)
    inp("ident", [P, P], F32)
    inp("m_lo", [P, cfg.TPC], BF16)
    inp("m_hi", [P, cfg.TPC], BF16)
    inp("srcw_lo", [P, cfg.TPC * cfg.NCHL_T * 8], I16)
    inp("srcw_hi", [P, cfg.TPC * cfg.NCHH_T * 8], I16)
    inp("s1idx_lo", [P, cfg.TPC * 8], I16)
    inp("s1idx_hi", [P, cfg.TPC * 8], I16)
    inp("S_A", [P, cfg.TPC * cfg.NCH_T * P], FP8)
    inp("ST_A", [P, cfg.TPC * cfg.NCH_T * P], FP8)
    t["table1"] = nc.dram_tensor("table1", [cfg.N_PAD, 256], BF16,
                                 kind="Internal").ap()
    t["table_s"] = nc.dram_tensor("table_s", [cfg.N_PAD, 128], BF16,
                                  kind="Internal").ap()
    t["table2slab"] = nc.dram_tensor("table2slab",
                                     [cfg.NPC_PAD, cfg.OUT + 2],
                                     F32, kind="ExternalOutput").ap()
    return t


def _decl_b(nc, cfg):
    t = {}

    def inp(name, shape, dt):
        t[name] = nc.dram_tensor(name, shape, dt, kind="ExternalInput").ap()

    inp("table2", [cfg.N_PAD, 64], BF16)
    inp("table_s2", [cfg.N_PAD, 128], BF16)
    inp("m_lo", [P, cfg.TPC], BF16)
    inp("m_hi", [P, cfg.TPC], BF16)
    inp("srcw_b", [P, cfg.TPC * cfg.NCB_T * 8], I16)
    inp("s1idx_lo", [P, cfg.TPC * 8], I16)
    inp("s1idx_hi", [P, cfg.TPC * 8], I16)
    inp("S_B", [P, cfg.TPC * cfg.NCB_T * P], FP8)
    inp("ST_B", [P, cfg.TPC * cfg.NCB_T * P], FP8)
    t["outp"] = nc.dram_tensor("outp", [cfg.NPC_PAD, cfg.OUT], F32,
                               kind="ExternalOutput").ap()
    return t


def _compile(build_fn, decl_fn, cfg):
    nc = bacc.Bacc("TRN2", target_bir_lowering=False, debug=False,
                   enable_asserts=False, num_devices=cfg.NCORES,
                   num_swdge_queues=NQ)
    t = decl_fn(nc, cfg)
    with tile.TileContext(nc) as tc:
        build_fn(tc, cfg, t)
    nc.compile()
    return nc


def _host_prep_weights(cfg, W1, att_src1, att_dst1, W2, att_src2, att_dst2):
    A_d1 = _blockdiag_att(np.asarray(att_dst1, np.float32), cfg.H, cfg.HID,
                          cfg.F)
    A_s1 = _blockdiag_att(np.asarray(att_src1, np.float32), cfg.H, cfg.HID,
                          cfg.F)
    W1T = np.asarray(W1, np.float32).T.copy()
    wpack = np.concatenate([W1T, W1T @ A_d1, W1T @ A_s1], axis=1)
    W2T = np.asarray(W2, np.float32).T.copy()
    a_d2 = np.asarray(att_dst2, np.float32).reshape(cfg.OUT, 1)
    a_s2 = np.asarray(att_src2, np.float32).reshape(cfg.OUT, 1)
    w2pack = np.concatenate([W2T, W2T @ a_d2, W2T @ a_s2], axis=1)
    return (np.ascontiguousarray(wpack.astype(BF)),
            np.ascontiguousarray(w2pack.astype(BF)))


_CACHE = {}


def _get_kernels(cfg):
    key = (cfg.N, cfg.E, cfg.NCORES, cfg.NCHL_T, cfg.NCHH_T,
           cfg.NCE_T, cfg.NCO_T)
    if key not in _CACHE:
        nca = _compile(_build_a, _decl_a, cfg)
        ncb = _compile(_build_b, _decl_b, cfg)
        _CACHE[key] = (nca, ncb)
    return _CACHE[key]


def run(cfg, inputs, runner=None):
    x = np.asarray(inputs["x"], np.float32)
    edge_index = np.asarray(inputs["edge_index"], np.int64)
    pc = _prep_graph(cfg, edge_index)
    wpack, w2pack = _host_prep_weights(
        cfg, inputs["W1"], inputs["att_src1"], inputs["att_dst1"],
        inputs["W2"], inputs["att_src2"], inputs["att_dst2"])

    xT = np.zeros((P, cfg.N_PAD), BF)
    xT[:, :cfg.N] = x.T.astype(BF)
    ident = np.eye(P, dtype=np.float32)

    nca, ncb = _get_kernels(cfg)

    if runner is None:
        def runner(nc, in_maps):
            r = bass_utils.run_bass_kernel_spmd(
                nc, in_maps, core_ids=list(range(cfg.NCORES)))
            return r.results

    in_maps_a = []
    for c in range(cfg.NCORES):
        in_maps_a.append(dict(
            xT=xT, wpack=wpack, w2pack=w2pack, ident=ident,
            m_lo=pc["m_lo"][c], m_hi=pc["m_hi"][c],
            srcw_lo=pc["srcw_lo"][c], srcw_hi=pc["srcw_hi"][c],
            s1idx_lo=pc["s1idx_lo"][c], s1idx_hi=pc["s1idx_hi"][c],
            S_A=pc["S_A"][c], ST_A=pc["ST_A"][c]))
    res_a = runner(nca, in_maps_a)

    # host gather: assemble layer-2 tables
    W2C = cfg.OUT + 2
    slab_all = np.zeros((cfg.N_PAD, W2C), np.float32)
    for c in range(cfg.NCORES):
        slab = np.asarray(res_a[c]["table2slab"], np.float32)
        slab_all[c * cfg.NPC:(c + 1) * cfg.NPC] = slab[:cfg.NPC]
    table2 = np.zeros((cfg.N_PAD, 64), BF)
    table2[:, 0:cfg.OUT] = slab_all[:, 0:cfg.OUT].astype(BF)
    table2[:, cfg.OUT] = slab_all[:, cfg.OUT].astype(BF)       # d2
    table_s2 = np.zeros((cfg.N_PAD, 128), BF)
    table_s2[:, 0] = slab_all[:, cfg.OUT + 1].astype(BF)       # s2

    in_maps_b = []
    for c in range(cfg.NCORES):
        in_maps_b.append(dict(
            table2=table2, table_s2=table_s2,
            m_lo=pc["m_lo"][c], m_hi=pc["m_hi"][c],
            srcw_b=pc["srcw_b"][c],
            s1idx_lo=pc["s1idx_lo"][c], s1idx_hi=pc["s1idx_hi"][c],
            S_B=pc["S_B"][c], ST_B=pc["ST_B"][c]))
    res_b = runner(ncb, in_maps_b)

    out = np.zeros((cfg.N, cfg.OUT), np.float32)
    for c in range(cfg.NCORES):
        out[c * cfg.NPC:(c + 1) * cfg.NPC] = \
            np.asarray(res_b[c]["outp"], np.float32)[:cfg.NPC]
    return out


def kernel(**inputs):
    cfg = Cfg(N=50000, E=1600000, ncores=8)
    return run(cfg, inputs)


# revision 13
# speedup vs baseline: 3.4533x; 1.2926x over previous
"""
2-layer GAT on Trainium2 (8 NeuronCores, SPMD via bass/Tile) — v2.

Sharding: destination nodes block-sharded across 8 cores (6250 each).
All per-edge work runs on the core owning the edge's dst.  Layer-0
node-level compute (h = x @ W1pack, bf16) is replicated on every core.
Two kernels (A: layer 1, B: layer 2) with a host gather of per-core
node tables in between.

Key performance structure (vs v1 baseline, 8.29 ms -> target ~1.3 ms):
  - 4 SWDGE queues (num_swdge_queues=4): dma_gather descriptor
    generation parallelises over all 4 Q7 cpu pairs (2.33 ns/idx vs
    7.92 measured).
  - ONE gather index per edge per layer: the dst-side attention scalar
    (s1/s2) is no longer gathered per edge.  Instead each dst tile
    fetches its 128 nodes' scalar with two tiny 128-idx gathers
    (lo/hi of the node id, masked combine), then broadcasts to edge
    slots with a per-chunk PE matmul  lhsT=ST (transposed one-hot).
  - One-hot matrices S (slot->dstlocal, aggregation lhsT) and ST
    (broadcast lhsT) are precomputed on host as fp8 and streamed.
  - Messages M are fp8 (keeps the big DVE mult in 1x mode: no
    GPSIMD/SWDGE port-pair lock), aggregation matmul fp8 x fp8.
  - Node phase in bf16 (fp32 matmul is 4x slower on PE).
"""

import os
import sys

import numpy as np
import ml_dtypes

for _p in ("/opt/trn_rl_repo",):
    if os.path.isdir(_p) and _p not in sys.path:
        sys.path.insert(0, _p)

import concourse.bass as bass
import concourse.bacc as bacc
import concourse.tile as tile
from concourse import mybir
from concourse import bass_utils
from concourse._compat import with_exitstack
from contextlib import ExitStack

F32 = mybir.dt.float32
BF16 = mybir.dt.bfloat16
FP8 = mybir.dt.float8e4
I32 = mybir.dt.int32
I16 = mybir.dt.int16
AF = mybir.ActivationFunctionType
OP = mybir.AluOpType
P = 128
BF = ml_dtypes.bfloat16
F8 = ml_dtypes.float8_e4m3
NQ = 4                     # SWDGE queues


class Cfg:
    def __init__(self, N, E, ncores, split=32768, neg=0.2, in_ch=128,
                 f=128, heads=8, hid=16, out=16):
        self.N = N
        self.E = E
        self.NCORES = ncores
        self.SPLIT = split
        self.NEG = neg
        self.IN = in_ch
        self.F = f
        self.H = heads
        self.HID = hid
        self.OUT = out
        assert N % ncores == 0
        self.NPC = N // ncores
        self.TPC = (self.NPC + P - 1) // P
        self.NPC_PAD = self.TPC * P
        self.NTILES = ncores * self.TPC
        self.N_PAD = self.NTILES * P
        # filled by _prep_graph
        self.NCHL_T = None   # lo chunks per tile (layer A)
        self.NCHH_T = None   # hi chunks per tile (layer A)
        self.NCH_T = None
        self.NCE_T = None    # even-src chunks per tile (layer B)
        self.NCO_T = None    # odd-src chunks per tile (layer B)
        self.NCB_T = None


def _wrap16(vals):
    """[n] slot-ordered int idx -> [128, n//16] int16 wrapped layout."""
    n = vals.shape[0]
    assert n % 16 == 0
    w = vals.reshape(-1, 16).T.astype(np.int16)
    return np.ascontiguousarray(np.tile(w, (8, 1)))


def _slot_fill(src_vals, dloc_vals, pos, nch_grp, grp_off, t, NCH_T, v_idx,
               s_rows, s_cols, st_rows, st_cols):
    """Record slot assignments for one (tile, group) run of edges."""
    chunk = t * NCH_T + grp_off + pos // P
    part = pos % P
    v_idx[...] = src_vals
    s_rows.append(part)
    s_cols.append(chunk * P + dloc_vals)
    st_rows.append(dloc_vals)
    st_cols.append(chunk * P + part)


def _prep_graph(cfg, edge_index):
    N, NPC, TPC, SPL = cfg.N, cfg.NPC, cfg.TPC, cfg.SPLIT
    src = np.concatenate([edge_index[0], np.arange(N, dtype=np.int64)])
    dst = np.concatenate([edge_index[1], np.arange(N, dtype=np.int64)])
    core = dst // NPC
    ld = dst - core * NPC
    tile_id = ld // P
    dloc = ld % P

    # ---- layer A grouping: (core, tile, hi(src), src) ----
    hi = (src >= SPL).astype(np.int64)
    orderA = np.lexsort((src, hi, tile_id, core))
    keyA = (core * TPC + tile_id) * 2 + hi
    cntA = np.bincount(keyA, minlength=cfg.NCORES * TPC * 2)
    cnt_lo = cntA[0::2].reshape(cfg.NCORES, TPC)
    cnt_hi = cntA[1::2].reshape(cfg.NCORES, TPC)
    cfg.NCHL_T = max(1, int(np.max((cnt_lo + P - 1) // P)))
    cfg.NCHH_T = max(1, int(np.max((cnt_hi + P - 1) // P)))
    cfg.NCH_T = cfg.NCHL_T + cfg.NCHH_T

    # ---- layer B grouping: (core, tile, parity(src), src) ----
    par = (src & 1).astype(np.int64)
    orderB = np.lexsort((src, par, tile_id, core))
    keyB = (core * TPC + tile_id) * 2 + par
    cntB = np.bincount(keyB, minlength=cfg.NCORES * TPC * 2)
    cnt_ev = cntB[0::2].reshape(cfg.NCORES, TPC)
    cnt_od = cntB[1::2].reshape(cfg.NCORES, TPC)
    cfg.NCE_T = max(1, int(np.max((cnt_ev + P - 1) // P)))
    cfg.NCO_T = max(1, int(np.max((cnt_od + P - 1) // P)))
    cfg.NCB_T = cfg.NCE_T + cfg.NCO_T

    startsA = np.concatenate([[0], np.cumsum(cntA)])
    startsB = np.concatenate([[0], np.cumsum(cntB)])
    ONE = np.uint8(0x38)  # 1.0 in float8_e4m3

    pc = dict(srcw_lo=[], srcw_hi=[], srcw_b=[], S_A=[], ST_A=[],
              S_B=[], ST_B=[], s1idx_lo=[], s1idx_hi=[], m_lo=[], m_hi=[])
    sA, dA = src[orderA], dloc[orderA]
    sB, dB = src[orderB], dloc[orderB]
    for c in range(cfg.NCORES):
        v_lo = np.zeros(TPC * cfg.NCHL_T * P, np.int64)
        v_hi = np.zeros(TPC * cfg.NCHH_T * P, np.int64)
        v_b = np.zeros(TPC * cfg.NCB_T * P, np.int64)
        SA = np.zeros((P, TPC * cfg.NCH_T * P), np.uint8)
        STA = np.zeros((P, TPC * cfg.NCH_T * P), np.uint8)
        SB = np.zeros((P, TPC * cfg.NCB_T * P), np.uint8)
        STB = np.zeros((P, TPC * cfg.NCB_T * P), np.uint8)
        for t in range(TPC):
            for g in (0, 1):
                # layer A
                k = (c * TPC + t) * 2 + g
                n = int(cntA[k])
                if n:
                    sl = slice(startsA[k], startsA[k] + n)
                    e_src, e_dl = sA[sl], dA[sl]
                    pos = np.arange(n)
                    if g == 0:
                        v_lo[t * cfg.NCHL_T * P + pos] = e_src
                        chunk = t * cfg.NCH_T + pos // P
                    else:
                        v_hi[t * cfg.NCHH_T * P + pos] = e_src - SPL
                        chunk = t * cfg.NCH_T + cfg.NCHL_T + pos // P
                    part = pos % P
                    SA[part, chunk * P + e_dl] = ONE
                    STA[e_dl, chunk * P + part] = ONE
                # layer B
                n = int(cntB[k])
                if n:
                    sl = slice(startsB[k], startsB[k] + n)
                    e_src, e_dl = sB[sl], dB[sl]
                    pos = np.arange(n)
                    if g == 0:
                        chunk = t * cfg.NCB_T + pos // P
                    else:
                        chunk = t * cfg.NCB_T + cfg.NCE_T + pos // P
                    v_b[chunk * P + pos % P] = e_src >> 1
                    part = pos % P
                    SB[part, chunk * P + e_dl] = ONE
                    STB[e_dl, chunk * P + part] = ONE
        pc["srcw_lo"].append(_wrap16(v_lo))
        pc["srcw_hi"].append(_wrap16(v_hi))
        pc["srcw_b"].append(_wrap16(v_b))
        pc["S_A"].append(SA.view(F8))
        pc["ST_A"].append(STA.view(F8))
        pc["S_B"].append(SB.view(F8))
        pc["ST_B"].append(STB.view(F8))
        # one batched dst-node s1 gather per core (lo/hi + mask combine);
        # slot i -> partition i%128, chunk i//128, so tile ti node j lands
        # at [j, ti, :]
        nodes = c * NPC + np.arange(cfg.NPC_PAD, dtype=np.int64)
        lo_sel = nodes < SPL
        idx_lo = np.where(lo_sel, nodes, 0)
        idx_hi = np.where(lo_sel, 0, nodes - SPL)
        pc["s1idx_lo"].append(_wrap16(idx_lo))
        pc["s1idx_hi"].append(_wrap16(idx_hi))
        m = lo_sel.astype(np.float32).reshape(TPC, P).T   # [128, TPC]
        pc["m_lo"].append(np.ascontiguousarray(m.astype(BF)))
        pc["m_hi"].append(np.ascontiguousarray((1.0 - m).astype(BF)))
    return pc


def _blockdiag_att(att, heads, hid, f):
    A = np.zeros((f, heads), dtype=np.float32)
    for h in range(heads):
        A[h * hid:(h + 1) * hid, h] = att[0, h]
    return A


def _ap(base, ap_list, off_extra=0):
    return bass.AP(tensor=base.tensor, offset=base.offset + off_extra,
                   ap=ap_list)


@with_exitstack
def _build_a(ctx, tc, cfg, t):
    nc = tc.nc
    NCHL_T, NCHH_T, NCH_T, TPC = cfg.NCHL_T, cfg.NCHH_T, cfg.NCH_T, cfg.TPC
    WCOLS = cfg.F + 2 * cfg.H             # 144 matmul out cols
    TCOLS = cfg.F + cfg.H                 # 136 table1 used cols
    MCOLS = cfg.F + cfg.H                 # 136 message cols
    ROW1 = 256                            # table1 row elems (bf16, 512B)
    ROWS = 128                            # table_s row elems (bf16, 256B)
    W2COLS = cfg.OUT + 2

    consts = ctx.enter_context(tc.tile_pool(name="consts", bufs=1))
    wpack = consts.tile([P, WCOLS], BF16)
    nc.sync.dma_start(out=wpack[:], in_=t["wpack"][:, :])
    w2pack = consts.tile([P, W2COLS], BF16)
    nc.sync.dma_start(out=w2pack[:], in_=t["w2pack"][:, :])
    ident = consts.tile([P, P], F32)
    nc.sync.dma_start(out=ident[:], in_=t["ident"][:, :])
    mlo = consts.tile([P, TPC], BF16)
    nc.sync.dma_start(out=mlo[:], in_=t["m_lo"][:, :])
    mhi = consts.tile([P, TPC], BF16)
    nc.sync.dma_start(out=mhi[:], in_=t["m_hi"][:, :])
    # all idx tiles loaded once
    il_all = consts.tile([P, TPC * NCHL_T * 8], I16)
    nc.sync.dma_start(out=il_all[:], in_=t["srcw_lo"][:, :])
    ih_all = consts.tile([P, TPC * NCHH_T * 8], I16)
    nc.sync.dma_start(out=ih_all[:], in_=t["srcw_hi"][:, :])
    isl_all = consts.tile([P, TPC * 8], I16)
    nc.sync.dma_start(out=isl_all[:], in_=t["s1idx_lo"][:, :])
    ish_all = consts.tile([P, TPC * 8], I16)
    nc.sync.dma_start(out=ish_all[:], in_=t["s1idx_hi"][:, :])

    # ---------------- node phase (bf16) ----------------
    NT = cfg.NTILES
    BLK = 8
    with ExitStack() as nctx:
        xpool = nctx.enter_context(tc.tile_pool(name="xt", bufs=2))
        npsum = nctx.enter_context(tc.tile_pool(name="npsum", bufs=4,
                                                space="PSUM"))
        nstage = nctx.enter_context(tc.tile_pool(name="nstage", bufs=3))
        for blk in range((NT + BLK - 1) // BLK):
            nt0 = blk * BLK
            nt1 = min(nt0 + BLK, NT)
            xt = xpool.tile([P, BLK * P], BF16, tag="xt")
            nc.sync.dma_start(out=xt[:, 0:(nt1 - nt0) * P],
                              in_=t["xT"][:, nt0 * P:nt1 * P])
            for j in range(nt1 - nt0):
                nt = nt0 + j
                pt = npsum.tile([P, WCOLS], F32, tag="npt")
                nc.tensor.matmul(out=pt[:], lhsT=xt[:, j * P:(j + 1) * P],
                                 rhs=wpack[:], start=True, stop=True)
                s1 = nstage.tile([P, TCOLS], BF16, tag="s1")
                nc.scalar.activation(s1[:], pt[:, 0:TCOLS], AF.Copy)
                ss = nstage.tile([P, cfg.H], BF16, tag="ss")
                nc.scalar.activation(ss[:], pt[:, TCOLS:WCOLS], AF.Copy)
                nc.sync.dma_start(
                    out=t["table1"][nt * P:(nt + 1) * P, 0:TCOLS], in_=s1[:])
                nc.sync.dma_start(
                    out=t["table_s"][nt * P:(nt + 1) * P, 0:cfg.H],
                    in_=ss[:])

    # Tile does not track DRAM deps: fence table writes vs gathers.
    tc.strict_bb_all_engine_barrier()

    # ---------------- edge phase ----------------
    gpool = ctx.enter_context(tc.tile_pool(name="g", bufs=3))
    spool = ctx.enter_context(tc.tile_pool(name="soh", bufs=6))
    slpool = ctx.enter_context(tc.tile_pool(name="sloc", bufs=4))
    lpool = ctx.enter_context(tc.tile_pool(name="logit", bufs=3))
    mpool = ctx.enter_context(tc.tile_pool(name="msg", bufs=3))
    lpsum = ctx.enter_context(tc.tile_pool(name="lpsum", bufs=2,
                                           space="PSUM"))
    apsum = ctx.enter_context(tc.tile_pool(name="apsum", bufs=2,
                                           space="PSUM"))
    tpsum = ctx.enter_context(tc.tile_pool(name="tpsum", bufs=2,
                                           space="PSUM"))
    t2psum = ctx.enter_context(tc.tile_pool(name="t2psum", bufs=2,
                                            space="PSUM"))
    hpool = ctx.enter_context(tc.tile_pool(name="h1", bufs=2))

    tab_hi = t["table1"][cfg.SPLIT:cfg.N_PAD, :]
    tabs_hi = t["table_s"][cfg.SPLIT:cfg.N_PAD, :]

    # one batched s1 fetch for all own dst nodes (2 gathers, masked merge)
    s1pool = ctx.enter_context(tc.tile_pool(name="s1all", bufs=1))
    s1Lo = s1pool.tile([P, TPC, ROWS], BF16)
    nc.gpsimd.dma_gather(
        out_ap=s1Lo[:], in_ap=t["table_s"][:, :], idxs_ap=isl_all[:],
        num_idxs=TPC * P, num_idxs_reg=TPC * P, elem_size=ROWS,
        single_packet=False, queue_num=2)
    s1Hi = s1pool.tile([P, TPC, ROWS], BF16)
    nc.gpsimd.dma_gather(
        out_ap=s1Hi[:], in_ap=tabs_hi, idxs_ap=ish_all[:],
        num_idxs=TPC * P, num_idxs_reg=TPC * P, elem_size=ROWS,
        single_packet=False, queue_num=3)

    for ti in range(TPC):
        q = [ti]

        def nextq():
            r = q[0] % NQ
            q[0] += 1
            return r
        # gathers: feat+d1 rows by src (lo/hi)
        G = gpool.tile([P, NCH_T, ROW1], BF16, tag="G")
        nc.gpsimd.dma_gather(
            out_ap=G[:, 0:NCHL_T, :], in_ap=t["table1"][:, :],
            idxs_ap=il_all[:, ti * NCHL_T * 8:(ti + 1) * NCHL_T * 8],
            num_idxs=NCHL_T * P, num_idxs_reg=NCHL_T * P,
            elem_size=ROW1, single_packet=False, queue_num=nextq())
        nc.gpsimd.dma_gather(
            out_ap=G[:, NCHL_T:NCH_T, :], in_ap=tab_hi,
            idxs_ap=ih_all[:, ti * NCHH_T * 8:(ti + 1) * NCHH_T * 8],
            num_idxs=NCHH_T * P, num_idxs_reg=NCHH_T * P,
            elem_size=ROW1, single_packet=False, queue_num=nextq())

        # one-hot streams
        S = spool.tile([P, NCH_T * P], FP8, tag="S")
        nc.sync.dma_start(out=S[:], in_=t["S_A"][
            :, ti * NCH_T * P:(ti + 1) * NCH_T * P])
        ST = spool.tile([P, NCH_T * P], FP8, tag="ST")
        nc.sync.dma_start(out=ST[:], in_=t["ST_A"][
            :, ti * NCH_T * P:(ti + 1) * NCH_T * P])

        # s1loc[j, :] = s1 of the tile's j-th node (lo/hi masked)
        sA = slpool.tile([P, cfg.H], BF16, tag="sA")
        nc.vector.tensor_tensor(
            out=sA[:], in0=s1Lo[:, ti, 0:cfg.H],
            in1=_ap(mlo[:], [mlo[:].ap[0], [0, cfg.H]], off_extra=ti),
            op=OP.mult)
        sB = slpool.tile([P, cfg.H], BF16, tag="sB")
        nc.vector.tensor_tensor(
            out=sB[:], in0=s1Hi[:, ti, 0:cfg.H],
            in1=_ap(mhi[:], [mhi[:].ap[0], [0, cfg.H]], off_extra=ti),
            op=OP.mult)
        s1loc = slpool.tile([P, cfg.H], BF16, tag="s1loc")
        nc.vector.tensor_tensor(out=s1loc[:], in0=sA[:], in1=sB[:],
                                op=OP.add)

        # broadcast s1[dst] to edge slots: psum_l[:, k, :] = ST_k.T @ s1loc
        psl = lpsum.tile([P, NCH_T, cfg.H], F32, tag="psl")
        for k in range(NCH_T):
            nc.tensor.matmul(out=psl[:, k, :],
                             lhsT=ST[:, k * P:(k + 1) * P],
                             rhs=s1loc[:], start=True, stop=True)

        # logits -> ex
        u = lpool.tile([P, NCH_T, cfg.H], BF16, tag="u")
        nc.vector.tensor_tensor(out=u[:], in0=psl[:],
                                in1=G[:, :, cfg.F:cfg.F + cfg.H], op=OP.add)
        a = lpool.tile([P, NCH_T, cfg.H], BF16, tag="a")
        nc.vector.scalar_tensor_tensor(out=a[:], in0=u[:], scalar=cfg.NEG,
                                       in1=u[:], op0=OP.mult, op1=OP.max)
        ex = lpool.tile([P, NCH_T, cfg.H], BF16, tag="ex")
        nc.scalar.activation(ex[:], a[:], AF.Exp)

        # M = [feat * ex | ex]  (fp8)
        M = mpool.tile([P, NCH_T, MCOLS], FP8, tag="M")
        nc.scalar.activation(M[:, :, cfg.F:MCOLS], ex[:], AF.Copy)
        nc.vector.tensor_tensor(
            out=_ap(M[:], [M[:].ap[0], [MCOLS, NCH_T], [cfg.HID, cfg.H],
                           [1, cfg.HID]]),
            in0=_ap(G[:], [G[:].ap[0], [ROW1, NCH_T], [cfg.HID, cfg.H],
                           [1, cfg.HID]]),
            in1=_ap(ex[:], [ex[:].ap[0], [cfg.H, NCH_T], [1, cfg.H],
                            [0, cfg.HID]]),
            op=OP.mult)

        # aggregate
        agg = apsum.tile([P, MCOLS], F32, tag="agg")
        for k in range(NCH_T):
            nc.tensor.matmul(out=agg[:], lhsT=S[:, k * P:(k + 1) * P],
                             rhs=M[:, k, :],
                             start=(k == 0), stop=(k == NCH_T - 1))

        # normalize + elu + feat2/d2/s2 slab
        den = hpool.tile([P, cfg.H], F32, tag="den")
        nc.vector.tensor_scalar_add(den[:], agg[:, cfg.F:MCOLS], 1e-20)
        rcp = hpool.tile([P, cfg.H], F32, tag="rcp")
        nc.vector.reciprocal(rcp[:], den[:])
        h1 = hpool.tile([P, cfg.F], F32, tag="h1")
        nc.vector.tensor_tensor(
            out=_ap(h1[:], [h1[:].ap[0], [cfg.HID, cfg.H], [1, cfg.HID]]),
            in0=_ap(agg[:], [agg[:].ap[0], [cfg.HID, cfg.H], [1, cfg.HID]]),
            in1=_ap(rcp[:], [rcp[:].ap[0], [1, cfg.H], [0, cfg.HID]]),
            op=OP.mult)
        pos = hpool.tile([P, cfg.F], F32, tag="pos")
        nc.scalar.activation(pos[:], h1[:], AF.Relu)
        nr = hpool.tile([P, cfg.F], F32, tag="nr")
        nc.scalar.activation(nr[:], h1[:], AF.Relu, scale=-1.0)
        een = hpool.tile([P, cfg.F], F32, tag="een")
        nc.scalar.activation(een[:], nr[:], AF.Exp, scale=-1.0)
        elu = hpool.tile([P, cfg.F], F32, tag="elu")
        nc.vector.scalar_tensor_tensor(out=elu[:], in0=een[:], scalar=-1.0,
                                       in1=pos[:], op0=OP.add, op1=OP.add)
        eT_ps = tpsum.tile([P, P], F32, tag="eT")
        nc.tensor.transpose(out=eT_ps[:], in_=elu[:], identity=ident[:])
        eT = hpool.tile([P, P], BF16, tag="eTs")
        nc.scalar.activation(eT[:], eT_ps[:], AF.Copy)
        t2 = t2psum.tile([P, W2COLS], F32, tag="t2")
        nc.tensor.matmul(out=t2[:], lhsT=eT[:], rhs=w2pack[:],
                         start=True, stop=True)
        t2s = hpool.tile([P, W2COLS], F32, tag="t2s")
        nc.scalar.activation(t2s[:], t2[:], AF.Copy)
        nc.sync.dma_start(out=t["table2slab"][ti * P:(ti + 1) * P, :],
                          in_=t2s[:])


@with_exitstack
def _build_b(ctx, tc, cfg, t):
    nc = tc.nc
    NCE_T, NCO_T, NCB_T, TPC = cfg.NCE_T, cfg.NCO_T, cfg.NCB_T, cfg.TPC
    UC = cfg.OUT + 1                    # 17 used row cols: feat2|d2
    MC = cfg.OUT + 1                    # 17 message cols
    ROW2 = 64                           # table2 row elems (bf16, 128B)

    consts = ctx.enter_context(tc.tile_pool(name="consts", bufs=1))
    ib_all = consts.tile([P, TPC * NCB_T * 8], I16)
    nc.sync.dma_start(out=ib_all[:], in_=t["srcw_b"][:, :])
    s2all = consts.tile([P, TPC], BF16)
    nc.sync.dma_start(out=s2all[:], in_=t["s2all"][:, :])

    gpool = ctx.enter_context(tc.tile_pool(name="g2", bufs=4))
    spool = ctx.enter_context(tc.tile_pool(name="soh2", bufs=8))
    lpool = ctx.enter_context(tc.tile_pool(name="l2", bufs=3))
    mpool = ctx.enter_context(tc.tile_pool(name="m2", bufs=3))
    lpsum = ctx.enter_context(tc.tile_pool(name="lps2", bufs=2,
                                           space="PSUM"))
    apsum = ctx.enter_context(tc.tile_pool(name="aps2", bufs=2,
                                           space="PSUM"))
    opool = ctx.enter_context(tc.tile_pool(name="o", bufs=3))
    tailp = ctx.enter_context(tc.tile_pool(name="tail", bufs=1))
    h2all = tailp.tile([P, TPC, cfg.OUT], F32)

    tab_pair = _ap(t["table2"][:, :], [[2 * ROW2, cfg.N_PAD // 2],
                                       [1, 2 * ROW2]])

    for ti in range(TPC):
        q = [ti]

        def nextq():
            r = q[0] % NQ
            q[0] += 1
            return r
        G = gpool.tile([P, NCB_T, 2 * ROW2], BF16, tag="G2")
        nc.gpsimd.dma_gather(
            out_ap=G[:], in_ap=tab_pair,
            idxs_ap=ib_all[:, ti * NCB_T * 8:(ti + 1) * NCB_T * 8],
            num_idxs=NCB_T * P, num_idxs_reg=NCB_T * P,
            elem_size=2 * ROW2, single_packet=False, queue_num=nextq())

        S = spool.tile([P, NCB_T * P], FP8, tag="SB")
        nc.sync.dma_start(out=S[:], in_=t["S_B"][
            :, ti * NCB_T * P:(ti + 1) * NCB_T * P])
        ST = spool.tile([P, NCB_T * P], FP8, tag="STB")
        nc.sync.dma_start(out=ST[:], in_=t["ST_B"][
            :, ti * NCB_T * P:(ti + 1) * NCB_T * P])

        psl = lpsum.tile([P, NCB_T, 1], F32, tag="psl2")
        for k in range(NCB_T):
            nc.tensor.matmul(out=psl[:, k, :],
                             lhsT=ST[:, k * P:(k + 1) * P],
                             rhs=s2all[:, ti:ti + 1], start=True, stop=True)

        # logits: u = s2[dst] + d2[src]; parity via static col offset
        u = lpool.tile([P, NCB_T, 1], BF16, tag="u2")
        nc.vector.tensor_tensor(
            out=u[:, 0:NCE_T, :], in0=psl[:, 0:NCE_T, :],
            in1=G[:, 0:NCE_T, cfg.OUT:cfg.OUT + 1], op=OP.add)
        nc.vector.tensor_tensor(
            out=u[:, NCE_T:NCB_T, :], in0=psl[:, NCE_T:NCB_T, :],
            in1=G[:, NCE_T:NCB_T, ROW2 + cfg.OUT:ROW2 + cfg.OUT + 1],
            op=OP.add)
        a = lpool.tile([P, NCB_T, 1], BF16, tag="a2")
        nc.vector.scalar_tensor_tensor(out=a[:], in0=u[:], scalar=cfg.NEG,
                                       in1=u[:], op0=OP.mult, op1=OP.max)
        ex = lpool.tile([P, NCB_T, 1], BF16, tag="ex2")
        nc.scalar.activation(ex[:], a[:], AF.Exp)

        M = mpool.tile([P, NCB_T, MC], FP8, tag="M2")
        nc.scalar.activation(M[:, :, cfg.OUT:MC], ex[:], AF.Copy)
        nc.vector.tensor_tensor(
            out=M[:, 0:NCE_T, 0:cfg.OUT],
            in0=G[:, 0:NCE_T, 0:cfg.OUT],
            in1=_ap(ex[:], [ex[:].ap[0], [1, NCE_T], [0, cfg.OUT]]),
            op=OP.mult)
        nc.vector.tensor_tensor(
            out=M[:, NCE_T:NCB_T, 0:cfg.OUT],
            in0=G[:, NCE_T:NCB_T, ROW2:ROW2 + cfg.OUT],
            in1=_ap(ex[:], [ex[:].ap[0], [1, NCO_T], [0, cfg.OUT]],
                    off_extra=NCE_T),
            op=OP.mult)

        agg = apsum.tile([P, MC], F32, tag="agg2")
        for k in range(NCB_T):
            nc.tensor.matmul(out=agg[:], lhsT=S[:, k * P:(k + 1) * P],
                             rhs=M[:, k, :],
                             start=(k == 0), stop=(k == NCB_T - 1))

        den = opool.tile([P, 1], F32, tag="den")
        nc.vector.tensor_scalar_add(den[:], agg[:, cfg.OUT:MC], 1e-20)
        rcp = opool.tile([P, 1], F32, tag="rcp")
        nc.vector.reciprocal(rcp[:], den[:])
        nc.vector.tensor_tensor(
            out=h2all[:, ti, :], in0=agg[:, 0:cfg.OUT],
            in1=_ap(rcp[:], [rcp[:].ap[0], [0, cfg.OUT]]), op=OP.mult)

    # batched log_softmax tail: one Exp + one Ln pass (avoids per-tile
    # activation-table reloads between Exp and Ln)
    m = tailp.tile([P, TPC], F32)
    nc.vector.tensor_reduce(out=m[:], in_=h2all[:],
                            axis=mybir.AxisListType.X, op=OP.max)
    tm = tailp.tile([P, TPC, cfg.OUT], F32)
    nc.vector.tensor_tensor(
        out=tm[:], in0=h2all[:],
        in1=_ap(m[:], [m[:].ap[0], [1, TPC], [0, cfg.OUT]]),
        op=OP.subtract)
    pe = tailp.tile([P, TPC, cfg.OUT], F32)
    nc.scalar.activation(pe[:], tm[:], AF.Exp)
    ssum = tailp.tile([P, TPC], F32)
    nc.vector.tensor_reduce(out=ssum[:], in_=pe[:],
                            axis=mybir.AxisListType.X, op=OP.add)
    ln = tailp.tile([P, TPC], F32)
    nc.scalar.activation(ln[:], ssum[:], AF.Ln)
    res = tailp.tile([P, TPC, cfg.OUT], F32)
    nc.vector.tensor_tensor(
        out=res[:], in0=tm[:],
        in1=_ap(ln[:], [ln[:].ap[0], [1, TPC], [0, cfg.OUT]]),
        op=OP.subtract)
    for ti in range(TPC):
        nc.sync.dma_start(out=t["outp"][ti * P:(ti + 1) * P, :],
                          in_=res[:, ti, :])


def _decl_a(nc, cfg):
    t = {}
    WCOLS = cfg.F + 2 * cfg.H
    W2COLS = cfg.OUT + 2

    def inp(name, shape, dt):
        t[name] = nc.dram_tensor(name, shape, dt, kind="ExternalInput").ap()

    inp("xT", [P, cfg.N_PAD], BF16)
    inp("wpack", [P, WCOLS], BF16)
    inp("w2pack", [P, W2COLS], BF16)
    inp("ident", [P, P], F32)
    inp("m_lo", [P, cfg.TPC], BF16)
    inp("m_hi", [P, cfg.TPC], BF16)
    inp("srcw_lo", [P, cfg.TPC * cfg.NCHL_T * 8], I16)
    inp("srcw_hi", [P, cfg.TPC * cfg.NCHH_T * 8], I16)
    inp("s1idx_lo", [P, cfg.TPC * 8], I16)
    inp("s1idx_hi", [P, cfg.TPC * 8], I16)
    inp("S_A", [P, cfg.TPC * cfg.NCH_T * P], FP8)
    inp("ST_A", [P, cfg.TPC * cfg.NCH_T * P], FP8)
    t["table1"] = nc.dram_tensor("table1", [cfg.N_PAD, 256], BF16,
                                 kind="Internal").ap()
    t["table_s"] = nc.dram_tensor("table_s", [cfg.N_PAD, 128], BF16,
                                  kind="Internal").ap()
    t["table2slab"] = nc.dram_tensor("table2slab",
                                     [cfg.NPC_PAD, cfg.OUT + 2],
                                     F32, kind="ExternalOutput").ap()
    return t


def _decl_b(nc, cfg):
    t = {}

    def inp(name, shape, dt):
        t[name] = nc.dram_tensor(name, shape, dt, kind="ExternalInput").ap()

    inp("table2", [cfg.N_PAD, 64], BF16)
    inp("s2all", [P, cfg.TPC], BF16)
    inp("srcw_b", [P, cfg.TPC * cfg.NCB_T * 8], I16)
    inp("S_B", [P, cfg.TPC * cfg.NCB_T * P], FP8)
    inp("ST_B", [P, cfg.TPC * cfg.NCB_T * P], FP8)
    t["outp"] = nc.dram_tensor("outp", [cfg.NPC_PAD, cfg.OUT], F32,
                               kind="ExternalOutput").ap()
    return t


def _compile(build_fn, decl_fn, cfg):
    nc = bacc.Bacc("TRN2", target_bir_lowering=False, debug=False,
                   enable_asserts=False, num_devices=cfg.NCORES,
                   num_swdge_queues=NQ)
    t = decl_fn(nc, cfg)
    with tile.TileContext(nc) as tc:
        build_fn(tc, cfg, t)
    nc.compile()
    return nc


def _host_prep_weights(cfg, W1, att_src1, att_dst1, W2, att_src2, att_dst2):
    A_d1 = _blockdiag_att(np.asarray(att_dst1, np.float32), cfg.H, cfg.HID,
                          cfg.F)
    A_s1 = _blockdiag_att(np.asarray(att_src1, np.float32), cfg.H, cfg.HID,
                          cfg.F)
    W1T = np.asarray(W1, np.float32).T.copy()
    wpack = np.concatenate([W1T, W1T @ A_d1, W1T @ A_s1], axis=1)
    W2T = np.asarray(W2, np.float32).T.copy()
    a_d2 = np.asarray(att_dst2, np.float32).reshape(cfg.OUT, 1)
    a_s2 = np.asarray(att_src2, np.float32).reshape(cfg.OUT, 1)
    w2pack = np.concatenate([W2T, W2T @ a_d2, W2T @ a_s2], axis=1)
    return (np.ascontiguousarray(wpack.astype(BF)),
            np.ascontiguousarray(w2pack.astype(BF)))


_CACHE = {}


def _get_kernels(cfg):
    key = (cfg.N, cfg.E, cfg.NCORES, cfg.NCHL_T, cfg.NCHH_T,
           cfg.NCE_T, cfg.NCO_T)
    if key not in _CACHE:
        nca = _compile(_build_a, _decl_a, cfg)
        ncb = _compile(_build_b, _decl_b, cfg)
        _CACHE[key] = (nca, ncb)
    return _CACHE[key]


def run(cfg, inputs, runner=None):
    x = np.asarray(inputs["x"], np.float32)
    edge_index = np.asarray(inputs["edge_index"], np.int64)
    pc = _prep_graph(cfg, edge_index)
    wpack, w2pack = _host_prep_weights(
        cfg, inputs["W1"], inputs["att_src1"], inputs["att_dst1"],
        inputs["W2"], inputs["att_src2"], inputs["att_dst2"])

    xT = np.zeros((P, cfg.N_PAD), BF)
    xT[:, :cfg.N] = x.T.astype(BF)
    ident = np.eye(P, dtype=np.float32)

    nca, ncb = _get_kernels(cfg)

    if runner is None:
        def runner(nc, in_maps):
            r = bass_utils.run_bass_kernel_spmd(
                nc, in_maps, core_ids=list(range(cfg.NCORES)))
            return r.results

    in_maps_a = []
    for c in range(cfg.NCORES):
        in_maps_a.append(dict(
            xT=xT, wpack=wpack, w2pack=w2pack, ident=ident,
            m_lo=pc["m_lo"][c], m_hi=pc["m_hi"][c],
            srcw_lo=pc["srcw_lo"][c], srcw_hi=pc["srcw_hi"][c],
            s1idx_lo=pc["s1idx_lo"][c], s1idx_hi=pc["s1idx_hi"][c],
            S_A=pc["S_A"][c], ST_A=pc["ST_A"][c]))
    res_a = runner(nca, in_maps_a)

    # host gather: assemble layer-2 tables
    W2C = cfg.OUT + 2
    slab_all = np.zeros((cfg.N_PAD, W2C), np.float32)
    for c in range(cfg.NCORES):
        slab = np.asarray(res_a[c]["table2slab"], np.float32)
        slab_all[c * cfg.NPC:(c + 1) * cfg.NPC] = slab[:cfg.NPC]
    table2 = np.zeros((cfg.N_PAD, 64), BF)
    table2[:, 0:cfg.OUT] = slab_all[:, 0:cfg.OUT].astype(BF)
    table2[:, cfg.OUT] = slab_all[:, cfg.OUT].astype(BF)       # d2
    s2_full = slab_all[:, cfg.OUT + 1].astype(BF)              # s2 per node

    in_maps_b = []
    for c in range(cfg.NCORES):
        s2all = np.ascontiguousarray(
            s2_full[c * cfg.NPC_PAD:(c + 1) * cfg.NPC_PAD]
            if False else
            s2_full[np.arange(cfg.NPC_PAD) + c * cfg.NPC].reshape(
                cfg.TPC, P).T.astype(BF))
        in_maps_b.append(dict(
            table2=table2, s2all=s2all, srcw_b=pc["srcw_b"][c],
            S_B=pc["S_B"][c], ST_B=pc["ST_B"][c]))
    res_b = runner(ncb, in_maps_b)

    out = np.zeros((cfg.N, cfg.OUT), np.float32)
    for c in range(cfg.NCORES):
        out[c * cfg.NPC:(c + 1) * cfg.NPC] = \
            np.asarray(res_b[c]["outp"], np.float32)[:cfg.NPC]
    return out


def kernel(**inputs):
    cfg = Cfg(N=50000, E=1600000, ncores=8)
    return run(cfg, inputs)
